# revision 10
# baseline (speedup 1.0000x reference)
"""Trainium2 Bass kernel for nn_MultiHeadNeuralMemoryMLP.

Math reformulation (per batch b, head n), avoiding the [L, L] decay masks:
  cum[l]      = cumsum(log_wd)[l],  wd_cross[l] = exp(cum[l])
  wd_inner[l, m] = wd_cross[m] / wd_cross[l]  (for l <= m)
  nG1 = -grad_Z1 * (lr * exp(-cum))[:, None]   (lr sign folded in)
  Z1_[m] = wd_cross[m] * (S_masked.T @ nG1 + X1_ @ W1.T)[m],  S = X1 @ X1_.T causal
  Z2_[m] = wd_cross[m] * (S2_masked.T @ nG2 + X2_ @ W2.T)[m], S2 = X2 @ X2_.T causal
  W1_next = wd_cross[L-1] * (W1 + nG1.T @ X1);  W2_next similarly.

Sharding: core = b * 4 + g handles batch b, heads 4g..4g+3. Projections use
replicated (sliced) weights; only the o-projection needs a cross-core sum,
done on the host over 4 partial [L, D] tensors per batch.
"""

import os
import sys

import numpy as np

if "/opt/trn_rl_repo" not in sys.path:
    sys.path.insert(0, "/opt/trn_rl_repo")

import concourse.bass as bass
import concourse.mybir as mybir
import concourse.tile as tile
from concourse import bacc
from concourse.bass_utils import run_bass_kernel_spmd

F32 = mybir.dt.float32
AF = mybir.ActivationFunctionType

B, L, D, NH, DH = 2, 1024, 1024, 16, 2048
HD, HDH = D // NH, DH // NH          # 64, 128
HPC = 4                               # heads per core
NCORES = 8
NLT = L // 128                        # 8 row tiles
NCH = L // 512                        # 2 column chunks


def build_program():
    nc = bacc.Bacc("TRN2", target_bir_lowering=False, debug=False,
                   num_devices=NCORES)

    # ---- DRAM I/O ----
    xT_d = nc.dram_tensor("xT", [D, L], F32, kind="ExternalInput")
    qkvT_d = nc.dram_tensor("qkvT", [D, 3 * HPC * HD], F32, kind="ExternalInput")
    qkvb_d = nc.dram_tensor("qkvb", [1, 3 * HPC * HD], F32, kind="ExternalInput")
    fcTs_d = nc.dram_tensor("fcTs", [128, 64], F32, kind="ExternalInput")
    fcb_d = nc.dram_tensor("fcb", [1, 8], F32, kind="ExternalInput")
    base_d = nc.dram_tensor("base", [128, 8], F32, kind="ExternalInput")
    w1t_d = nc.dram_tensor("w1t", [HPC, HD, HDH], F32, kind="ExternalInput")
    w1n_d = nc.dram_tensor("w1n", [HPC, HDH, HD], F32, kind="ExternalInput")
    w2t_d = nc.dram_tensor("w2t", [HPC, HDH, HD], F32, kind="ExternalInput")
    w2n_d = nc.dram_tensor("w2n", [HPC, HD, HDH], F32, kind="ExternalInput")
    owT_d = nc.dram_tensor("owT", [HPC * HD, D], F32, kind="ExternalInput")
    obrow_d = nc.dram_tensor("obrow", [1, D], F32, kind="ExternalInput")

    out_d = nc.dram_tensor("out_p", [L, D], F32, kind="ExternalOutput")
    w1next_d = nc.dram_tensor("w1next", [HPC, HDH, HD], F32, kind="ExternalOutput")
    w2next_d = nc.dram_tensor("w2next", [HPC, HD, HDH], F32, kind="ExternalOutput")

    triu_d = nc.inline_tensor(np.triu(np.ones((128, 128), np.float32)), "triu")
    ident_d = nc.inline_tensor(np.eye(128, dtype=np.float32), "ident")
    ones_d = nc.inline_tensor(np.ones((128, 512), np.float32), "ones")

    with tile.TileContext(nc) as tc:
        with (
            tc.tile_pool(name="big", bufs=8) as big,       # [128,1024] slots
            tc.tile_pool(name="bigq", bufs=8) as bigq,     # [128,768] slots
            tc.tile_pool(name="qkv", bufs=1) as qkvp,      # persists Q/K/V
            tc.tile_pool(name="const", bufs=1) as cst,
            tc.tile_pool(name="scal", bufs=1) as scp,      # stage-A persistents
            tc.tile_pool(name="head", bufs=8) as hp,       # per-lt per-head tiles
            tc.tile_pool(name="head2", bufs=2) as hp2,     # per-head tiles
            tc.tile_pool(name="stile", bufs=8) as sp,      # S tiles
            tc.tile_pool(name="tmp", bufs=4) as tmp,
            tc.tile_pool(name="psA", bufs=3, space="PSUM") as psA,   # [128,512]
            tc.tile_pool(name="psT", bufs=1, space="PSUM") as psT,   # held accum
            tc.tile_pool(name="psB", bufs=2, space="PSUM") as psB,   # [128,128]
            tc.tile_pool(name="psW", bufs=1, space="PSUM") as psW,
        ):
            dma = nc.sync.dma_start

            # ---- constants & inputs to SBUF ----
            triu = cst.tile([128, 128], F32, tag="triu")
            ident = cst.tile([128, 128], F32, tag="ident")
            ones = cst.tile([128, 512], F32, tag="ones")
            dma(triu[:], triu_d[:])
            dma(ident[:], ident_d[:])
            dma(ones[:], ones_d[:])

            xt = []
            for kt in range(NLT):
                t = big.tile([128, L], F32, tag="big")
                dma(t[:], xT_d[kt * 128:(kt + 1) * 128, :])
                xt.append(t)
            qk = []
            for kt in range(NLT):
                t = bigq.tile([128, 3 * HPC * HD], F32, tag="bigq")
                dma(t[:], qkvT_d[kt * 128:(kt + 1) * 128, :])
                qk.append(t)
            fcTs = cst.tile([128, 64], F32, tag="fcTs")
            fcb = cst.tile([1, 8], F32, tag="fcb")
            base_s = cst.tile([128, 8], F32, tag="base")
            qkvb = cst.tile([1, 3 * HPC * HD], F32, tag="qkvb")
            obrow = cst.tile([1, D], F32, tag="obrow")
            dma(fcTs[:], fcTs_d[:])
            dma(fcb[:], fcb_d[:])
            dma(base_s[:], base_d[:])
            dma(qkvb[:], qkvb_d[:])
            dma(obrow[:], obrow_d[:])
            owt = []
            for kt in range(2):
                t = big.tile([128, D], F32, tag="big", name=f"owt{kt}")
                dma(t[:], owT_d[kt * 128:(kt + 1) * 128, :])
                owt.append(t)

            # ---- stage A: fc projections, log_wd, cumsum, per-l scalars ----
            sb = []     # sig * base  [128, 8]: cols 0:4 = -lr, 4:8 = wd arg
            lwp = []    # log_wd padded to cols 0/32/64/96  [128, 128]
            for lt in range(NLT):
                psfc = psB.tile([128, 8], F32, tag="psB")
                for kt in range(NLT):
                    nc.tensor.matmul(
                        psfc[:], xt[kt][:, lt * 128:(lt + 1) * 128],
                        fcTs[:, kt * 8:kt * 8 + 8],
                        start=(kt == 0), stop=False)
                nc.tensor.matmul(psfc[:], ones[0:1, 0:128], fcb[:],
                                 start=False, stop=True)
                sig = tmp.tile([128, 8], F32, tag="sig")
                nc.scalar.activation(sig[:], psfc[:], AF.Sigmoid)
                sbt = scp.tile([128, 8], F32, tag=f"sb{lt}")
                nc.vector.tensor_mul(sbt[:], sig[:], base_s[:])
                sb.append(sbt)
                lw = scp.tile([128, 128], F32, tag=f"lwp{lt}")
                nc.vector.memset(lw[:], 0.0)
                for n in range(HPC):
                    nc.scalar.activation(
                        lw[:, 32 * n:32 * n + 1], sbt[:, 4 + n:5 + n],
                        AF.Ln, bias=1.0, scale=-1.0)
                lwp.append(lw)

            # cumT (T layout, heads at partitions 0/32/64/96)
            cumTp = scp.tile([128, L], F32, tag="cumTp")
            for mt in range(NLT):
                pscum = psB.tile([128, 128], F32, tag="psB")
                for lt in range(mt):
                    nc.tensor.matmul(pscum[:], lwp[lt][:], ones[0:128, 0:128],
                                     start=(lt == 0), stop=False)
                nc.tensor.matmul(pscum[:], lwp[mt][:], triu[:],
                                 start=(mt == 0), stop=True)
                nc.vector.tensor_copy(cumTp[:, mt * 128:(mt + 1) * 128], pscum[:])
            wdcTp = scp.tile([128, L], F32, tag="wdcTp")
            nc.scalar.activation(wdcTp[:], cumTp[:], AF.Exp)

            # wd_last broadcast [128, 4] (col n = wd_cross[L-1] of head n)
            wdlast = scp.tile([128, HPC], F32, tag="wdlast")
            for n in range(HPC):
                pswl = psB.tile([128, 1], F32, tag="psB")
                nc.tensor.matmul(pswl[:], ones[32 * n:32 * n + 1, 0:128],
                                 wdcTp[32 * n:32 * n + 1, L - 1:L],
                                 tile_position=(32 * n, 0))
                nc.vector.tensor_copy(wdlast[:, n:n + 1], pswl[:])

            # natural-layout lrw[lt][:, n] = -lr[l] * exp(-cum[l])
            lrw = []
            for lt in range(NLT):
                psct = psB.tile([128, 128], F32, tag="psB")
                nc.tensor.transpose(psct[:], cumTp[:, lt * 128:(lt + 1) * 128],
                                    ident[:])
                lw_t = scp.tile([128, HPC], F32, tag=f"lrw{lt}")
                for n in range(HPC):
                    iw = tmp.tile([128, 1], F32, tag="iw")
                    nc.scalar.activation(iw[:], psct[:, 32 * n:32 * n + 1],
                                         AF.Exp, scale=-1.0)
                    nc.vector.tensor_mul(lw_t[:, n:n + 1], iw[:],
                                         sb[lt][:, n:n + 1])
                lrw.append(lw_t)

            # ---- stage B: q/k/v projections in T layout ----
            qkv_t = []   # [Q2T(2), K2T(2), V2T(2)]
            for j in range(3):
                pair_tiles = []
                for mg in range(2):
                    dst = qkvp.tile([128, L], F32, tag=f"qkv{j}{mg}")
                    off = j * HPC * HD + mg * 128
                    for ch in range(NCH):
                        psp = psA.tile([128, 512], F32, tag="psA")
                        for kt in range(NLT):
                            nc.tensor.matmul(
                                psp[:], qk[kt][:, off:off + 128],
                                xt[kt][:, ch * 512:(ch + 1) * 512],
                                start=(kt == 0), stop=False)
                        nc.tensor.matmul(psp[:], qkvb[0:1, off:off + 128],
                                         ones[0:1, 0:512], start=False, stop=True)
                        if ch == 0:
                            nc.scalar.copy(dst[:, ch * 512:(ch + 1) * 512], psp[:])
                        else:
                            nc.vector.tensor_copy(dst[:, ch * 512:(ch + 1) * 512],
                                                  psp[:])
                    pair_tiles.append(dst)
                qkv_t.append(pair_tiles)
            Q2T, K2T, V2T = qkv_t

            # Z2catT: pair tile kt holds heads 2kt, 2kt+1 (rows 0:64 / 64:128)
            z2cat = [big.tile([128, L], F32, tag="big", name=f"z2cat{i}")
                     for i in range(2)]

            # ---- stage C/D: heads processed in pairs (even head at
            # partition base 0, odd at 64 -> K=64 matmuls row-pack) ----
            for pair in range(2):
                heads = [2 * pair, 2 * pair + 1]
                st_ = {}     # per-head forward state

                for n in heads:
                    bp = (n % 2) * 64
                    X1T = K2T[pair][bp:bp + 64, :]
                    VT = V2T[pair][bp:bp + 64, :]

                    w1t_s = hp2.tile([128, HDH], F32, tag="w1t")
                    dma(w1t_s[0:64, :], w1t_d[n])
                    dma(w1t_s[64:128, :], w1t_d[n])
                    w1n_s = hp2.tile([HDH, HD], F32, tag="w1n")
                    dma(w1n_s[:], w1n_d[n])
                    w2t_s = hp2.tile([HDH, HD], F32, tag="w2t")
                    dma(w2t_s[:], w2t_d[n])
                    w2n_s = hp2.tile([HD, HDH], F32, tag="w2n")
                    dma(w2n_s[:], w2n_d[n])

                    # forward
                    x2n, dsl = [], []
                    X2T = big.tile([128, L], F32, tag="big", name=f"X2T{n}")
                    for lt in range(NLT):
                        c0, c1 = lt * 128, (lt + 1) * 128
                        ps1 = psB.tile([128, HDH], F32, tag="psB")
                        nc.tensor.matmul(ps1[:], X1T[:, c0:c1],
                                         w1t_s[bp:bp + 64, :])
                        xa = hp.tile([128, HDH], F32, tag="x2n", bufs=16)
                        nc.scalar.activation(xa[:], ps1[:], AF.Silu)
                        da = hp.tile([128, HDH], F32, tag="dsl", bufs=16)
                        nc.scalar.activation(da[:], ps1[:], AF.Derivative_silu)
                        x2n.append(xa)
                        dsl.append(da)
                        ps2 = psB.tile([128, 128], F32, tag="psB")
                        nc.tensor.transpose(ps2[:], xa[:], ident[:])
                        nc.vector.tensor_copy(X2T[:, c0:c1], ps2[:])

                    gZ2T = hp2.tile([HD, L], F32, tag="gz2t")
                    for ch in range(NCH):
                        ps3 = psA.tile([HD, 512], F32, tag="psA")
                        nc.tensor.matmul(ps3[:], w2t_s[:],
                                         X2T[:, ch * 512:(ch + 1) * 512])
                        nc.vector.tensor_sub(gZ2T[:, ch * 512:(ch + 1) * 512],
                                             ps3[:], VT[:, ch * 512:(ch + 1) * 512])

                    nG1, nG2, X1n = [], [], []
                    for lt in range(NLT):
                        c0, c1 = lt * 128, (lt + 1) * 128
                        ps4 = psB.tile([128, HDH], F32, tag="psB")
                        nc.tensor.matmul(ps4[:], gZ2T[:, c0:c1], w2n_s[:])
                        t1 = tmp.tile([128, HDH], F32, tag="t1")
                        nc.vector.tensor_mul(t1[:], ps4[:], dsl[lt][:])
                        g1 = hp.tile([128, HDH], F32, tag="ng1", bufs=16)
                        nc.vector.tensor_scalar_mul(g1[:], t1[:],
                                                    lrw[lt][:, n:n + 1])
                        nG1.append(g1)
                        ps5 = psB.tile([128, HD], F32, tag="psB")
                        nc.tensor.transpose(ps5[:], gZ2T[:, c0:c1],
                                            ident[0:64, 0:64])
                        g2 = hp.tile([128, HD], F32, tag="ng2", bufs=16)
                        nc.vector.tensor_scalar_mul(g2[:], ps5[:],
                                                    lrw[lt][:, n:n + 1])
                        nG2.append(g2)
                        ps6 = psB.tile([128, HD], F32, tag="psB")
                        nc.tensor.transpose(ps6[:], X1T[:, c0:c1],
                                            ident[bp:bp + 64, bp:bp + 64])
                        x1 = hp.tile([128, HD], F32, tag="x1n", bufs=16)
                        nc.vector.tensor_copy(x1[:], ps6[:])
                        X1n.append(x1)

                    # weight updates
                    psw1 = psW.tile([HDH, HD], F32, tag="psW")
                    for lt in range(NLT):
                        nc.tensor.matmul(psw1[:], nG1[lt][:], X1n[lt][:],
                                         start=(lt == 0), stop=(lt == NLT - 1))
                    tw1 = tmp.tile([HDH, HD], F32, tag="tw1")
                    nc.vector.tensor_add(tw1[:], psw1[:], w1n_s[:])
                    nc.vector.tensor_scalar_mul(tw1[:], tw1[:], wdlast[:, n:n + 1])
                    dma(w1next_d[n], tw1[:])
                    psw2 = psW.tile([HD, HDH], F32, tag="psW")
                    for lt in range(NLT):
                        nc.tensor.matmul(psw2[:], nG2[lt][:], x2n[lt][:],
                                         start=(lt == 0), stop=(lt == NLT - 1))
                    tw2 = tmp.tile([HD, HDH], F32, tag="tw2")
                    nc.vector.tensor_add(tw2[:], psw2[:], w2n_s[0:64, :])
                    nc.vector.tensor_scalar_mul(tw2[:], tw2[:],
                                                wdlast[0:64, n:n + 1])
                    dma(w2next_d[n], tw2[:])

                    st_[n] = dict(
                        X2T=X2T, nG1=nG1, nG2=nG2, w1t_s=w1t_s, w2t_s=w2t_s,
                        X2_T=big.tile([128, L], F32, tag="big", name=f"X2_T{n}"))

                # readout: interleave the two heads per 512-chunk
                for ch in range(NCH):
                    m0, m1 = ch * 512, (ch + 1) * 512
                    mb0 = 4 * ch
                    nlts = 4 * ch + 4

                    for n in heads:
                        s = st_[n]
                        bp = (n % 2) * 64
                        X1T = K2T[pair][bp:bp + 64, :]
                        X1_T = Q2T[pair][bp:bp + 64, :]

                        wdb = hp2.tile([128, 512], F32, tag="wdb", bufs=4)
                        pswb = psA.tile([128, 512], F32, tag="psA")
                        nc.tensor.matmul(pswb[:], ones[32 * n:32 * n + 1, 0:128],
                                         wdcTp[32 * n:32 * n + 1, m0:m1],
                                         tile_position=(32 * n, 0))
                        nc.vector.tensor_copy(wdb[:], pswb[:])
                        s["wdb"] = wdb

                        S = []
                        for lt in range(nlts):
                            psS = psA.tile([128, 512], F32, tag="psA")
                            nc.tensor.matmul(psS[:],
                                             X1T[:, lt * 128:(lt + 1) * 128],
                                             X1_T[:, m0:m1])
                            st = sp.tile([128, 512], F32, tag="s", bufs=16)
                            if lt < mb0:
                                if lt % 2 == 0:
                                    nc.scalar.copy(st[:], psS[:])
                                else:
                                    nc.vector.tensor_copy(st[:], psS[:])
                            else:
                                j0 = (lt - mb0) * 128
                                if j0 > 0:
                                    nc.vector.memset(st[:, 0:j0], 0.0)
                                nc.vector.tensor_mul(st[:, j0:j0 + 128],
                                                     psS[:, j0:j0 + 128], triu[:])
                                if j0 + 128 < 512:
                                    nc.scalar.copy(st[:, j0 + 128:512],
                                                   psS[:, j0 + 128:512])
                            S.append(st)

                        psT1 = psT.tile([128, 512], F32, tag="psT", bufs=2)
                        for lt in range(nlts):
                            nc.tensor.matmul(psT1[:], s["nG1"][lt][:], S[lt][:],
                                             start=(lt == 0), stop=False)
                        nc.tensor.matmul(psT1[:], s["w1t_s"][bp:bp + 64, :],
                                         X1_T[:, m0:m1], start=False, stop=True)
                        z1t = hp2.tile([128, 512], F32, tag="z1t", bufs=4)
                        nc.vector.tensor_mul(z1t[:], psT1[:], s["wdb"][:])
                        nc.scalar.activation(s["X2_T"][:, m0:m1], z1t[:], AF.Silu)

                    for n in heads:
                        s = st_[n]
                        bp = (n % 2) * 64

                        S2 = []
                        for lt in range(nlts):
                            psS2 = psA.tile([128, 512], F32, tag="psA")
                            nc.tensor.matmul(psS2[:],
                                             s["X2T"][:, lt * 128:(lt + 1) * 128],
                                             s["X2_T"][:, m0:m1])
                            st = bigq.tile([128, 512], F32, tag="bigq")
                            if lt < mb0:
                                if lt % 2 == 0:
                                    nc.vector.tensor_copy(st[:], psS2[:])
                                else:
                                    nc.scalar.copy(st[:], psS2[:])
                            else:
                                j0 = (lt - mb0) * 128
                                if j0 > 0:
                                    nc.vector.memset(st[:, 0:j0], 0.0)
                                nc.vector.tensor_mul(st[:, j0:j0 + 128],
                                                     psS2[:, j0:j0 + 128],
                                                     triu[:])
                                if j0 + 128 < 512:
                                    nc.scalar.copy(st[:, j0 + 128:512],
                                                   psS2[:, j0 + 128:512])
                            S2.append(st)

                        psT2 = psT.tile([HD, 512], F32, tag="psT", bufs=2)
                        for lt in range(nlts):
                            nc.tensor.matmul(psT2[:], s["nG2"][lt][:], S2[lt][:],
                                             start=(lt == 0), stop=False)
                        nc.tensor.matmul(psT2[:], s["w2t_s"][:],
                                         s["X2_T"][:, m0:m1],
                                         start=False, stop=True)
                        nc.vector.tensor_mul(z2cat[pair][bp:bp + 64, m0:m1],
                                             psT2[:], s["wdb"][0:64, :])

            # ---- stage E: o-projection (partial, heads of this core) ----
            for lt in range(NLT):
                outs = big.tile([128, D], F32, tag="big")
                for ch in range(NCH):
                    psO = psA.tile([128, 512], F32, tag="psA")
                    for kt in range(2):
                        nc.tensor.matmul(
                            psO[:], z2cat[kt][:, lt * 128:(lt + 1) * 128],
                            owt[kt][:, ch * 512:(ch + 1) * 512],
                            start=(kt == 0), stop=False)
                    nc.tensor.matmul(psO[:], ones[0:1, 0:128],
                                     obrow[0:1, ch * 512:(ch + 1) * 512],
                                     start=False, stop=True)
                    if ch == 0:
                        nc.scalar.copy(outs[:, ch * 512:(ch + 1) * 512], psO[:])
                    else:
                        nc.vector.tensor_copy(outs[:, ch * 512:(ch + 1) * 512],
                                              psO[:])
                dma(out_d[lt * 128:(lt + 1) * 128, :], outs[:])

    nc.compile()
    return nc


_NC = None


def _get_nc():
    global _NC
    if _NC is None:
        _NC = build_program()
    return _NC


def make_in_maps(x, W1, W2, log_base_lr, fc_lr_w, fc_lr_b, log_base_weight_decay,
                 fc_wd_w, fc_wd_b, q_w, q_b, k_w, k_b, v_w, v_b, o_w, o_b):
    f = np.float32
    in_maps = []
    for core in range(NCORES):
        b, g = core // 4, core % 4
        hs = slice(g * HPC, (g + 1) * HPC)          # global head ids
        cs = slice(g * HPC * HD, (g + 1) * HPC * HD)  # model-dim cols (256)
        xT = np.ascontiguousarray(x[b].T, dtype=f)
        qkvT = np.ascontiguousarray(
            np.concatenate([q_w[cs, :].T, k_w[cs, :].T, v_w[cs, :].T], axis=1),
            dtype=f)
        qkvb = np.concatenate([q_b[cs], k_b[cs], v_b[cs]])[None, :].astype(f)
        # fc: [1024, 8] -> packed [128, 64] (col block 8k = d-rows 128k..)
        fcT = np.concatenate([fc_lr_w[hs, :].T, fc_wd_w[hs, :].T], axis=1)  # [1024, 8]
        fcTs = np.ascontiguousarray(
            fcT.reshape(8, 128, 8).transpose(1, 0, 2).reshape(128, 64), dtype=f)
        fcb = np.concatenate([fc_lr_b[hs], fc_wd_b[hs]])[None, :].astype(f)
        base = np.concatenate([-np.exp(log_base_lr[hs]),
                               np.exp(log_base_weight_decay[hs])])
        base = np.broadcast_to(base[None, :], (128, 8)).astype(f)
        w1 = W1[b, hs]                                # [4, 128, 64]
        w2 = W2[b, hs]                                # [4, 64, 128]
        w1t = np.ascontiguousarray(w1.transpose(0, 2, 1), dtype=f)
        w2t = np.ascontiguousarray(w2.transpose(0, 2, 1), dtype=f)
        owT = np.ascontiguousarray(o_w[:, cs].T, dtype=f)
        obrow = (o_b if g == 0 else np.zeros_like(o_b))[None, :].astype(f)
        in_maps.append({
            "xT": xT, "qkvT": qkvT, "qkvb": qkvb, "fcTs": fcTs, "fcb": fcb,
            "base": np.ascontiguousarray(base),
            "w1t": w1t, "w1n": np.ascontiguousarray(w1, dtype=f),
            "w2t": w2t, "w2n": np.ascontiguousarray(w2, dtype=f),
            "owT": owT, "obrow": obrow,
        })
    return in_maps


def run(inputs, trace=False):
    nc = _get_nc()
    in_maps = make_in_maps(**inputs)
    res = run_bass_kernel_spmd(nc, in_maps, list(range(NCORES)), trace=trace)
    out = np.zeros((B, L, D), np.float32)
    W1n = np.zeros((B, NH, HDH, HD), np.float32)
    W2n = np.zeros((B, NH, HD, HDH), np.float32)
    for core in range(NCORES):
        b, g = core // 4, core % 4
        r = res.results[core]
        out[b] += r["out_p"]
        W1n[b, g * HPC:(g + 1) * HPC] = r["w1next"]
        W2n[b, g * HPC:(g + 1) * HPC] = r["w2next"]
    return (out, W1n, W2n), res


def kernel(**inputs):
    inputs = {k: np.asarray(v) for k, v in inputs.items()}
    (out, W1n, W2n), _ = run(inputs)
    return out, W1n, W2n


if __name__ == "__main__":
    print("building program...")
    nc = _get_nc()
    print("built ok")


# revision 16
# speedup vs baseline: 2.2896x; 2.2896x over previous
"""Trainium2 Bass kernel for nn_MultiHeadNeuralMemoryMLP.

Math reformulation (per batch b, head n), avoiding the [L, L] decay masks:
  cum[l]      = cumsum(log_wd)[l],  wd_cross[l] = exp(cum[l])
  wd_inner[l, m] = wd_cross[m] / wd_cross[l]  (for l <= m)
  nG1 = -grad_Z1 * (lr * exp(-cum))[:, None]   (lr sign folded in)
  Z1_[m] = wd_cross[m] * (S_masked.T @ nG1 + X1_ @ W1.T)[m],  S = X1 @ X1_.T causal
  Z2_[m] = wd_cross[m] * (S2_masked.T @ nG2 + X2_ @ W2.T)[m], S2 = X2 @ X2_.T causal
  W1_next = wd_cross[L-1] * (W1 + nG1.T @ X1);  W2_next similarly.

Sharding: core = b * 4 + g handles batch b, heads 4g..4g+3. Projections use
replicated (sliced) weights; only the o-projection needs a cross-core sum,
done on the host over 4 partial [L, D] tensors per batch.

Matmul operands are bf16 (f32 PSUM accumulation); the scalar pipeline
(fc projections, log-weight-decay cumsum, per-token scalars) stays f32.
The scalar engine runs only Sigmoid in steady state (silu/dsilu composed
on DVE) to avoid activation-table reloads.
"""

import sys

import numpy as np
import ml_dtypes

if "/opt/trn_rl_repo" not in sys.path:
    sys.path.insert(0, "/opt/trn_rl_repo")

import concourse.bass as bass
import concourse.mybir as mybir
import concourse.tile as tile
from concourse import bacc
from concourse.bass_utils import run_bass_kernel_spmd

F32 = mybir.dt.float32
BF16 = mybir.dt.bfloat16
AF = mybir.ActivationFunctionType
BFNP = ml_dtypes.bfloat16

B, L, D, NH, DH = 2, 1024, 1024, 16, 2048
HD, HDH = D // NH, DH // NH          # 64, 128
HPC = 4                               # heads per core
NCORES = 8
NLT = L // 128                        # 8 row tiles
NCH = L // 512                        # 2 column chunks


def build_program():
    nc = bacc.Bacc("TRN2", target_bir_lowering=False, debug=False,
                   num_devices=NCORES)

    # ---- DRAM I/O ----
    xT_d = nc.dram_tensor("xT", [D, L], F32, kind="ExternalInput")
    xTb_d = nc.dram_tensor("xTb", [D, L], BF16, kind="ExternalInput")
    qkvT_d = nc.dram_tensor("qkvT", [D, 3 * HPC * HD], BF16, kind="ExternalInput")
    qkvb_d = nc.dram_tensor("qkvb", [1, 3 * HPC * HD], BF16, kind="ExternalInput")
    fcTs_d = nc.dram_tensor("fcTs", [128, 64], F32, kind="ExternalInput")
    fcb_d = nc.dram_tensor("fcb", [1, 8], F32, kind="ExternalInput")
    base_d = nc.dram_tensor("base", [128, 8], F32, kind="ExternalInput")
    w1t_d = nc.dram_tensor("w1t", [HPC, HD, HDH], BF16, kind="ExternalInput")
    w1n_d = nc.dram_tensor("w1n", [HPC, HDH, HD], F32, kind="ExternalInput")
    w2t_d = nc.dram_tensor("w2t", [HPC, HDH, HD], BF16, kind="ExternalInput")
    w2nb_d = nc.dram_tensor("w2nb", [HPC, HD, HDH], BF16, kind="ExternalInput")
    w2n_d = nc.dram_tensor("w2n", [HPC, HD, HDH], F32, kind="ExternalInput")
    owT_d = nc.dram_tensor("owT", [HPC * HD, D], BF16, kind="ExternalInput")
    obrow_d = nc.dram_tensor("obrow", [1, D], BF16, kind="ExternalInput")

    out_d = nc.dram_tensor("out_p", [L, D], F32, kind="ExternalOutput")
    w1next_d = nc.dram_tensor("w1next", [HPC, HDH, HD], F32, kind="ExternalOutput")
    w2next_d = nc.dram_tensor("w2next", [HPC, HD, HDH], F32, kind="ExternalOutput")

    triu_d = nc.inline_tensor(np.triu(np.ones((128, 128), np.float32)), "triu")
    ident_d = nc.inline_tensor(np.eye(128, dtype=np.float32), "ident")
    identb_d = nc.inline_tensor(np.eye(128, dtype=BFNP), "identb")
    ones_d = nc.inline_tensor(np.ones((128, 512), np.float32), "ones")
    onesb_d = nc.inline_tensor(np.ones((128, 512), BFNP), "onesb")

    with tile.TileContext(nc) as tc:
        with (
            tc.tile_pool(name="big", bufs=8) as big,       # 4KB/partition slots
            tc.tile_pool(name="bigq", bufs=8) as bigq,
            tc.tile_pool(name="qkv", bufs=1) as qkvp,
            tc.tile_pool(name="const", bufs=1) as cst,
            tc.tile_pool(name="scal", bufs=1) as scp,
            tc.tile_pool(name="head", bufs=8) as hp,
            tc.tile_pool(name="head2", bufs=2) as hp2,
            tc.tile_pool(name="stile", bufs=16) as sp,
            tc.tile_pool(name="tmp", bufs=2) as tmp,
            tc.tile_pool(name="psA", bufs=3, space="PSUM") as psA,
            tc.tile_pool(name="psT", bufs=2, space="PSUM") as psT,
            tc.tile_pool(name="psB", bufs=1, space="PSUM") as psB,
            tc.tile_pool(name="psW", bufs=1, space="PSUM") as psW,
        ):
            dma = nc.sync.dma_start

            # ---- constants & inputs to SBUF ----
            triu = cst.tile([128, 128], F32, tag="triu")
            ident = cst.tile([128, 128], F32, tag="ident")
            identb = cst.tile([128, 128], BF16, tag="identb")
            ones = cst.tile([128, 512], F32, tag="ones")
            onesb = cst.tile([128, 512], BF16, tag="onesb")
            dma(triu[:], triu_d[:])
            dma(ident[:], ident_d[:])
            dma(identb[:], identb_d[:])
            dma(ones[:], ones_d[:])
            dma(onesb[:], onesb_d[:])

            xt, xtb, qk = [], [], []
            for kt in range(NLT):
                t = big.tile([128, L], F32, tag="big", name=f"xt{kt}")
                dma(t[:], xT_d[kt * 128:(kt + 1) * 128, :])
                xt.append(t)
                tb = bigq.tile([128, L], BF16, tag="xtb", name=f"xtb{kt}")
                dma(tb[:], xTb_d[kt * 128:(kt + 1) * 128, :])
                xtb.append(tb)
                tq = bigq.tile([128, 3 * HPC * HD], BF16, tag="qk",
                               name=f"qk{kt}")
                dma(tq[:], qkvT_d[kt * 128:(kt + 1) * 128, :])
                qk.append(tq)
            fcTs = cst.tile([128, 64], F32, tag="fcTs")
            fcb = cst.tile([1, 8], F32, tag="fcb")
            base_s = cst.tile([128, 8], F32, tag="base")
            qkvb = cst.tile([1, 3 * HPC * HD], BF16, tag="qkvb")
            obrow = cst.tile([1, D], BF16, tag="obrow")
            dma(fcTs[:], fcTs_d[:])
            dma(fcb[:], fcb_d[:])
            dma(base_s[:], base_d[:])
            dma(qkvb[:], qkvb_d[:])
            dma(obrow[:], obrow_d[:])
            owt = []
            for kt in range(2):
                t = bigq.tile([128, D], BF16, tag="owt", name=f"owt{kt}",
                              bufs=2)
                dma(t[:], owT_d[kt * 128:(kt + 1) * 128, :])
                owt.append(t)

            # ---- stage A: fc projections, log_wd, cumsum, per-l scalars ----
            sb = []     # sig * base  [128, 8]: cols 0:4 = -lr, 4:8 = wd arg
            lwp = []    # log_wd padded to cols 0/32/64/96  [128, 128]
            for lt in range(NLT):
                psfc = psB.tile([128, 8], F32, tag="psB")
                for kt in range(NLT):
                    nc.tensor.matmul(
                        psfc[:], xt[kt][:, lt * 128:(lt + 1) * 128],
                        fcTs[:, kt * 8:kt * 8 + 8],
                        start=(kt == 0), stop=False)
                nc.tensor.matmul(psfc[:], ones[0:1, 0:128], fcb[:],
                                 start=False, stop=True)
                sig = tmp.tile([128, 8], F32, tag="sig")
                nc.scalar.activation(sig[:], psfc[:], AF.Sigmoid)
                sbt = scp.tile([128, 8], F32, tag=f"sb{lt}")
                nc.vector.tensor_mul(sbt[:], sig[:], base_s[:])
                sb.append(sbt)
                lw = scp.tile([128, 128], F32, tag=f"lwp{lt}")
                nc.vector.memset(lw[:], 0.0)
                for n in range(HPC):
                    nc.scalar.activation(
                        lw[:, 32 * n:32 * n + 1], sbt[:, 4 + n:5 + n],
                        AF.Ln, bias=1.0, scale=-1.0)
                lwp.append(lw)

            # cumT (T layout, heads at partitions 0/32/64/96)
            cumTp = scp.tile([128, L], F32, tag="cumTp")
            for mt in range(NLT):
                pscum = psB.tile([128, 128], F32, tag="psB")
                for lt in range(mt):
                    nc.tensor.matmul(pscum[:], lwp[lt][:], ones[0:128, 0:128],
                                     start=(lt == 0), stop=False)
                nc.tensor.matmul(pscum[:], lwp[mt][:], triu[:],
                                 start=(mt == 0), stop=True)
                nc.vector.tensor_copy(cumTp[:, mt * 128:(mt + 1) * 128], pscum[:])
            wdcTp = scp.tile([128, L], F32, tag="wdcTp")
            nc.scalar.activation(wdcTp[:], cumTp[:], AF.Exp)

            # wd_last broadcast [128, 4] (col n = wd_cross[L-1] of head n)
            wdlast = scp.tile([128, HPC], F32, tag="wdlast")
            for n in range(HPC):
                pswl = psB.tile([128, 1], F32, tag="psB")
                nc.tensor.matmul(pswl[:], ones[32 * n:32 * n + 1, 0:128],
                                 wdcTp[32 * n:32 * n + 1, L - 1:L],
                                 tile_position=(32 * n, 0))
                nc.vector.tensor_copy(wdlast[:, n:n + 1], pswl[:])

            # natural-layout lrw[lt][:, n] = -lr[l] * exp(-cum[l])
            lrw = []
            for lt in range(NLT):
                psct = psB.tile([128, 128], F32, tag="psB")
                nc.tensor.transpose(psct[:], cumTp[:, lt * 128:(lt + 1) * 128],
                                    ident[:])
                lw_t = scp.tile([128, HPC], F32, tag=f"lrw{lt}")
                for n in range(HPC):
                    iw = tmp.tile([128, 1], F32, tag="iw")
                    nc.scalar.activation(iw[:], psct[:, 32 * n:32 * n + 1],
                                         AF.Exp, scale=-1.0)
                    nc.vector.tensor_mul(lw_t[:, n:n + 1], iw[:],
                                         sb[lt][:, n:n + 1])
                lrw.append(lw_t)

            # ---- stage B: q/k/v projections in T layout (bf16) ----
            qkv_t = []
            for j in range(3):
                pair_tiles = []
                for mg in range(2):
                    dst = qkvp.tile([128, L], BF16, tag=f"qkv{j}{mg}")
                    off = j * HPC * HD + mg * 128
                    for ch in range(NCH):
                        psp = psA.tile([128, 512], F32, tag="psA")
                        for kt in range(NLT):
                            nc.tensor.matmul(
                                psp[:], qk[kt][:, off:off + 128],
                                xtb[kt][:, ch * 512:(ch + 1) * 512],
                                start=(kt == 0), stop=False)
                        nc.tensor.matmul(psp[:], qkvb[0:1, off:off + 128],
                                         onesb[0:1, 0:512], start=False, stop=True)
                        nc.vector.tensor_copy(dst[:, ch * 512:(ch + 1) * 512],
                                              psp[:])
                    pair_tiles.append(dst)
                qkv_t.append(pair_tiles)
            Q2T, K2T, V2T = qkv_t

            # Z2catT: pair tile kt holds heads 2kt, 2kt+1 (rows 0:64 / 64:128)
            z2cat = [big.tile([128, L], BF16, tag="big", name=f"z2cat{i}")
                     for i in range(2)]

            # ---- stage C/D: heads processed in pairs ----
            for pair in range(2):
                heads = [2 * pair, 2 * pair + 1]
                st_ = {}

                for n in heads:
                    bp = (n % 2) * 64
                    X1T = K2T[pair][bp:bp + 64, :]
                    VT = V2T[pair][bp:bp + 64, :]

                    w1t_s = hp2.tile([128, HDH], BF16, tag="w1t")
                    dma(w1t_s[0:64, :], w1t_d[n])
                    dma(w1t_s[64:128, :], w1t_d[n])
                    w1n_s = hp2.tile([HDH, HD], F32, tag="w1n")
                    dma(w1n_s[:], w1n_d[n])
                    w2t_s = hp2.tile([HDH, HD], BF16, tag="w2t")
                    dma(w2t_s[:], w2t_d[n])
                    w2nb_s = hp2.tile([HD, HDH], BF16, tag="w2nb")
                    dma(w2nb_s[:], w2nb_d[n])
                    w2n_s = hp2.tile([HD, HDH], F32, tag="w2n")
                    dma(w2n_s[:], w2n_d[n])

                    # forward, packed in 512-col halves (4 l-tiles each)
                    x2nh, dslh = [], []
                    X2T = big.tile([128, L], BF16, tag="big", name=f"X2T{n}")
                    for hf in range(2):
                        h0 = hf * 512
                        psZ = psA.tile([128, 512], F32, tag="psA")
                        for j in range(4):
                            c0 = h0 + j * 128
                            nc.tensor.matmul(psZ[:, j * 128:(j + 1) * 128],
                                             X1T[:, c0:c0 + 128],
                                             w1t_s[bp:bp + 64, :])
                        sg = tmp.tile([128, 512], F32, tag="sg")
                        nc.scalar.activation(sg[:], psZ[:], AF.Sigmoid)
                        z1s = tmp.tile([128, 512], F32, tag="z1s")
                        nc.vector.tensor_copy(z1s[:], psZ[:])
                        xa = hp.tile([128, 512], BF16, tag="x2n", bufs=4)
                        nc.vector.tensor_mul(xa[:], z1s[:], sg[:])   # silu
                        x2nh.append(xa)
                        # dsilu = sil + sg*(1 - sil)
                        w_ = tmp.tile([128, 512], F32, tag="w_")
                        nc.vector.tensor_scalar(w_[:], xa[:], -1.0, 1.0,
                                                mybir.AluOpType.mult,
                                                mybir.AluOpType.add)
                        da = hp.tile([128, 512], F32, tag="dsl", bufs=4)
                        nc.vector.scalar_tensor_tensor(
                            da[:], sg[:], 1.0, w_[:],
                            mybir.AluOpType.mult, mybir.AluOpType.mult)
                        nc.vector.tensor_add(da[:], da[:], xa[:])
                        dslh.append(da)
                        psX = psA.tile([128, 512], BF16, tag="psA")
                        for j in range(4):
                            nc.tensor.transpose(psX[:, j * 128:(j + 1) * 128],
                                                xa[:, j * 128:(j + 1) * 128],
                                                identb[:])
                        nc.vector.tensor_copy(X2T[:, h0:h0 + 512], psX[:])

                    gZ2T = hp2.tile([HD, L], BF16, tag="gz2t")
                    for ch in range(NCH):
                        ps3 = psA.tile([HD, 512], F32, tag="psA")
                        nc.tensor.matmul(ps3[:], w2t_s[:],
                                         X2T[:, ch * 512:(ch + 1) * 512])
                        nc.vector.tensor_sub(gZ2T[:, ch * 512:(ch + 1) * 512],
                                             ps3[:], VT[:, ch * 512:(ch + 1) * 512])

                    nG1h, nG2h, X1nh = [], [], []
                    for hf in range(2):
                        h0 = hf * 512
                        psG = psA.tile([128, 512], F32, tag="psA")
                        for j in range(4):
                            c0 = h0 + j * 128
                            nc.tensor.matmul(psG[:, j * 128:(j + 1) * 128],
                                             gZ2T[:, c0:c0 + 128], w2nb_s[:])
                        t1 = tmp.tile([128, 512], F32, tag="t1")
                        nc.vector.tensor_mul(t1[:], psG[:], dslh[hf][:])
                        g1 = hp.tile([128, 512], BF16, tag="ng1", bufs=4)
                        for j in range(4):
                            lt = hf * 4 + j
                            nc.vector.tensor_scalar_mul(
                                g1[:, j * 128:(j + 1) * 128],
                                t1[:, j * 128:(j + 1) * 128],
                                lrw[lt][:, n:n + 1])
                        nG1h.append(g1)
                        psU = psA.tile([128, 256], BF16, tag="psA",
                                       name="psU")
                        for j in range(4):
                            c0 = h0 + j * 128
                            nc.tensor.transpose(psU[:, j * 64:(j + 1) * 64],
                                                gZ2T[:, c0:c0 + 128],
                                                identb[0:64, 0:64])
                        g2 = hp.tile([128, 256], BF16, tag="ng2", bufs=4)
                        for j in range(4):
                            lt = hf * 4 + j
                            nc.vector.tensor_scalar_mul(
                                g2[:, j * 64:(j + 1) * 64],
                                psU[:, j * 64:(j + 1) * 64],
                                lrw[lt][:, n:n + 1])
                        nG2h.append(g2)
                        psV = psA.tile([128, 256], BF16, tag="psA",
                                       name="psV")
                        for j in range(4):
                            c0 = h0 + j * 128
                            nc.tensor.transpose(psV[:, j * 64:(j + 1) * 64],
                                                X1T[:, c0:c0 + 128],
                                                identb[bp:bp + 64, bp:bp + 64])
                        x1 = hp.tile([128, 256], BF16, tag="x1n", bufs=4)
                        nc.vector.tensor_copy(x1[:], psV[:])
                        X1nh.append(x1)

                    # weight updates
                    psw1 = psW.tile([HDH, HD], F32, tag="psW")
                    for lt in range(NLT):
                        hf, j = lt // 4, lt % 4
                        nc.tensor.matmul(psw1[:],
                                         nG1h[hf][:, j * 128:(j + 1) * 128],
                                         X1nh[hf][:, j * 64:(j + 1) * 64],
                                         start=(lt == 0), stop=(lt == NLT - 1))
                    tw1 = tmp.tile([HDH, HD], F32, tag="tw1")
                    nc.vector.tensor_add(tw1[:], psw1[:], w1n_s[:])
                    nc.vector.tensor_scalar_mul(tw1[:], tw1[:], wdlast[:, n:n + 1])
                    dma(w1next_d[n], tw1[:])
                    psw2 = psW.tile([HD, HDH], F32, tag="psW")
                    for lt in range(NLT):
                        hf, j = lt // 4, lt % 4
                        nc.tensor.matmul(psw2[:],
                                         nG2h[hf][:, j * 64:(j + 1) * 64],
                                         x2nh[hf][:, j * 128:(j + 1) * 128],
                                         start=(lt == 0), stop=(lt == NLT - 1))
                    tw2 = tmp.tile([HD, HDH], F32, tag="tw2")
                    nc.vector.tensor_add(tw2[:], psw2[:], w2n_s[0:64, :])
                    nc.vector.tensor_scalar_mul(tw2[:], tw2[:],
                                                wdlast[0:64, n:n + 1])
                    dma(w2next_d[n], tw2[:])

                    st_[n] = dict(
                        X2T=X2T, nG1h=nG1h, nG2h=nG2h, w1t_s=w1t_s, w2t_s=w2t_s,
                        X2_T=big.tile([128, L], BF16, tag="big", name=f"X2_T{n}"))

                # readout: interleave the two heads per 512-chunk
                for ch in range(NCH):
                    m0, m1 = ch * 512, (ch + 1) * 512
                    mb0 = 4 * ch
                    nlts = 4 * ch + 4

                    for n in heads:
                        s = st_[n]
                        bp = (n % 2) * 64
                        X1T = K2T[pair][bp:bp + 64, :]
                        X1_T = Q2T[pair][bp:bp + 64, :]

                        wdb = hp2.tile([128, 512], F32, tag="wdb", bufs=4)
                        pswb = psA.tile([128, 512], F32, tag="psA")
                        nc.tensor.matmul(pswb[:], ones[32 * n:32 * n + 1, 0:128],
                                         wdcTp[32 * n:32 * n + 1, m0:m1],
                                         tile_position=(32 * n, 0))
                        nc.vector.tensor_copy(wdb[:], pswb[:])
                        s["wdb"] = wdb

                        S = []
                        for lt in range(nlts):
                            j0 = 0 if lt < mb0 else (lt - mb0) * 128
                            psS = psA.tile([128, 512], F32, tag="psA")
                            nc.tensor.matmul(psS[:, j0:512],
                                             X1T[:, lt * 128:(lt + 1) * 128],
                                             X1_T[:, m0 + j0:m1])
                            st = sp.tile([128, 512], BF16, tag="s")
                            if lt < mb0:
                                if lt % 2 == 0:
                                    nc.scalar.copy(st[:], psS[:])
                                else:
                                    nc.vector.tensor_copy(st[:], psS[:])
                            else:
                                nc.vector.tensor_mul(st[:, j0:j0 + 128],
                                                     psS[:, j0:j0 + 128], triu[:])
                                if j0 + 128 < 512:
                                    nc.scalar.copy(st[:, j0 + 128:512],
                                                   psS[:, j0 + 128:512])
                            S.append(st)

                        psT1 = psT.tile([128, 512], F32, tag="psT")
                        for lt in range(nlts):
                            j0 = 0 if lt < mb0 else (lt - mb0) * 128
                            nc.tensor.matmul(psT1[:, j0:512],
                                             s["nG1h"][lt // 4]
                                             [:, (lt % 4) * 128:(lt % 4 + 1) * 128],
                                             S[lt][:, j0:512],
                                             start=(lt == 0), stop=False)
                        nc.tensor.matmul(psT1[:], s["w1t_s"][bp:bp + 64, :],
                                         X1_T[:, m0:m1], start=False, stop=True)
                        z1t = hp2.tile([128, 512], F32, tag="z1t", bufs=4)
                        nc.vector.tensor_mul(z1t[:], psT1[:], s["wdb"][:])
                        sgro = tmp.tile([128, 512], F32, tag="sgro")
                        nc.scalar.activation(sgro[:], z1t[:], AF.Sigmoid)
                        nc.vector.tensor_mul(s["X2_T"][:, m0:m1], z1t[:], sgro[:])

                    for n in heads:
                        s = st_[n]
                        bp = (n % 2) * 64

                        S2 = []
                        for lt in range(nlts):
                            j0 = 0 if lt < mb0 else (lt - mb0) * 128
                            psS2 = psA.tile([128, 512], F32, tag="psA")
                            nc.tensor.matmul(psS2[:, j0:512],
                                             s["X2T"][:, lt * 128:(lt + 1) * 128],
                                             s["X2_T"][:, m0 + j0:m1])
                            st = sp.tile([128, 512], BF16, tag="s", name="s2t")
                            if lt < mb0:
                                if lt % 2 == 0:
                                    nc.vector.tensor_copy(st[:], psS2[:])
                                else:
                                    nc.scalar.copy(st[:], psS2[:])
                            else:
                                nc.vector.tensor_mul(st[:, j0:j0 + 128],
                                                     psS2[:, j0:j0 + 128],
                                                     triu[:])
                                if j0 + 128 < 512:
                                    nc.scalar.copy(st[:, j0 + 128:512],
                                                   psS2[:, j0 + 128:512])
                            S2.append(st)

                        psT2 = psT.tile([HD, 512], F32, tag="psT")
                        for lt in range(nlts):
                            j0 = 0 if lt < mb0 else (lt - mb0) * 128
                            nc.tensor.matmul(psT2[:, j0:512],
                                             s["nG2h"][lt // 4]
                                             [:, (lt % 4) * 64:(lt % 4 + 1) * 64],
                                             S2[lt][:, j0:512],
                                             start=(lt == 0), stop=False)
                        nc.tensor.matmul(psT2[:], s["w2t_s"][:],
                                         s["X2_T"][:, m0:m1],
                                         start=False, stop=True)
                        nc.vector.tensor_mul(z2cat[pair][bp:bp + 64, m0:m1],
                                             psT2[:], s["wdb"][0:64, :])

            # ---- stage E: o-projection (partial, heads of this core) ----
            for lt in range(NLT):
                outs = big.tile([128, D], F32, tag="big", name=f"outs{lt}")
                for ch in range(NCH):
                    psO = psA.tile([128, 512], F32, tag="psA")
                    for kt in range(2):
                        nc.tensor.matmul(
                            psO[:], z2cat[kt][:, lt * 128:(lt + 1) * 128],
                            owt[kt][:, ch * 512:(ch + 1) * 512],
                            start=(kt == 0), stop=False)
                    nc.tensor.matmul(psO[:], onesb[0:1, 0:128],
                                     obrow[0:1, ch * 512:(ch + 1) * 512],
                                     start=False, stop=True)
                    nc.vector.tensor_copy(outs[:, ch * 512:(ch + 1) * 512],
                                          psO[:])
                dma(out_d[lt * 128:(lt + 1) * 128, :], outs[:])

    nc.compile()
    return nc


_NC = None


def _get_nc():
    global _NC
    if _NC is None:
        _NC = build_program()
    return _NC


def make_in_maps(x, W1, W2, log_base_lr, fc_lr_w, fc_lr_b, log_base_weight_decay,
                 fc_wd_w, fc_wd_b, q_w, q_b, k_w, k_b, v_w, v_b, o_w, o_b):
    f = np.float32
    in_maps = []
    for core in range(NCORES):
        b, g = core // 4, core % 4
        hs = slice(g * HPC, (g + 1) * HPC)
        cs = slice(g * HPC * HD, (g + 1) * HPC * HD)
        xT = np.ascontiguousarray(x[b].T, dtype=f)
        qkvT = np.ascontiguousarray(
            np.concatenate([q_w[cs, :].T, k_w[cs, :].T, v_w[cs, :].T], axis=1),
            dtype=BFNP)
        qkvb = np.concatenate([q_b[cs], k_b[cs], v_b[cs]])[None, :].astype(BFNP)
        fcT = np.concatenate([fc_lr_w[hs, :].T, fc_wd_w[hs, :].T], axis=1)
        fcTs = np.ascontiguousarray(
            fcT.reshape(8, 128, 8).transpose(1, 0, 2).reshape(128, 64), dtype=f)
        fcb = np.concatenate([fc_lr_b[hs], fc_wd_b[hs]])[None, :].astype(f)
        base = np.concatenate([-np.exp(log_base_lr[hs]),
                               np.exp(log_base_weight_decay[hs])])
        base = np.broadcast_to(base[None, :], (128, 8)).astype(f)
        w1 = W1[b, hs]
        w2 = W2[b, hs]
        w1t = np.ascontiguousarray(w1.transpose(0, 2, 1), dtype=BFNP)
        w2t = np.ascontiguousarray(w2.transpose(0, 2, 1), dtype=BFNP)
        owT = np.ascontiguousarray(o_w[:, cs].T, dtype=BFNP)
        obrow = (o_b if g == 0 else np.zeros_like(o_b))[None, :].astype(BFNP)
        in_maps.append({
            "xT": xT, "xTb": xT.astype(BFNP), "qkvT": qkvT, "qkvb": qkvb,
            "fcTs": fcTs, "fcb": fcb, "base": np.ascontiguousarray(base),
            "w1t": w1t, "w1n": np.ascontiguousarray(w1, dtype=f),
            "w2t": w2t, "w2nb": np.ascontiguousarray(w2, dtype=BFNP),
            "w2n": np.ascontiguousarray(w2, dtype=f),
            "owT": owT, "obrow": obrow,
        })
    return in_maps


def run(inputs, trace=False):
    nc = _get_nc()
    in_maps = make_in_maps(**inputs)
    res = run_bass_kernel_spmd(nc, in_maps, list(range(NCORES)), trace=trace)
    out = np.zeros((B, L, D), np.float32)
    W1n = np.zeros((B, NH, HDH, HD), np.float32)
    W2n = np.zeros((B, NH, HD, HDH), np.float32)
    for core in range(NCORES):
        b, g = core // 4, core % 4
        r = res.results[core]
        out[b] += r["out_p"]
        W1n[b, g * HPC:(g + 1) * HPC] = r["w1next"]
        W2n[b, g * HPC:(g + 1) * HPC] = r["w2next"]
    return (out, W1n, W2n), res


def kernel(**inputs):
    inputs = {k: np.asarray(v) for k, v in inputs.items()}
    (out, W1n, W2n), _ = run(inputs)
    return out, W1n, W2n


if __name__ == "__main__":
    print("building program...")
    nc = _get_nc()
    print("built ok")


# revision 29
# speedup vs baseline: 2.3999x; 1.0482x over previous
"""Trainium2 Bass kernel for nn_MultiHeadNeuralMemoryMLP.

Math reformulation (per batch b, head n), avoiding the [L, L] decay masks:
  cum[l]      = cumsum(log_wd)[l],  wd_cross[l] = exp(cum[l])
  wd_inner[l, m] = wd_cross[m] / wd_cross[l]  (for l <= m)
  nG1 = -grad_Z1 * (lr * exp(-cum))[:, None]   (lr sign folded in)
  Z1_[m] = wd_cross[m] * (S_masked.T @ nG1 + X1_ @ W1.T)[m],  S = X1 @ X1_.T causal
  Z2_[m] = wd_cross[m] * (S2_masked.T @ nG2 + X2_ @ W2.T)[m], S2 = X2 @ X2_.T causal
  W1_next = wd_cross[L-1] * (W1 + nG1.T @ X1);  W2_next similarly.

Sharding: core = b * 4 + g handles batch b, heads 4g..4g+3. Projections use
replicated (sliced) weights; only the o-projection needs a cross-core sum,
done on the host over 4 partial [L, D] tensors per batch.

Matmul operands are bf16 (f32 PSUM accumulation); the scalar pipeline
(fc projections, log-weight-decay cumsum, per-token scalars) stays f32.
The scalar engine runs only Sigmoid in steady state (silu/dsilu composed
on DVE) to avoid activation-table reloads.
"""

import sys

import numpy as np
import ml_dtypes

if "/opt/trn_rl_repo" not in sys.path:
    sys.path.insert(0, "/opt/trn_rl_repo")

import concourse.bass as bass
import concourse.mybir as mybir
import concourse.tile as tile
from concourse import bacc
from concourse.bass_utils import run_bass_kernel_spmd

F32 = mybir.dt.float32
BF16 = mybir.dt.bfloat16
AF = mybir.ActivationFunctionType
BFNP = ml_dtypes.bfloat16

B, L, D, NH, DH = 2, 1024, 1024, 16, 2048
HD, HDH = D // NH, DH // NH          # 64, 128
HPC = 4                               # heads per core
NCORES = 8
NLT = L // 128                        # 8 row tiles
NCH = L // 512                        # 2 column chunks


def build_program():
    nc = bacc.Bacc("TRN2", target_bir_lowering=False, debug=False,
                   num_devices=NCORES)

    # ---- DRAM I/O ----
    xT_d = nc.dram_tensor("xT", [D, L], F32, kind="ExternalInput")
    xTb_d = nc.dram_tensor("xTb", [D, L], BF16, kind="ExternalInput")
    qkvT_d = nc.dram_tensor("qkvT", [D, 3 * HPC * HD], BF16, kind="ExternalInput")
    fcp_d = nc.dram_tensor("fcp", [128, L], F32, kind="ExternalInput")
    fcbp_d = nc.dram_tensor("fcbp", [1, 128], F32, kind="ExternalInput")
    basep_d = nc.dram_tensor("basep", [128, 1], F32, kind="ExternalInput")
    qkvbT_d = nc.dram_tensor("qkvbT", [128, 6], F32, kind="ExternalInput")
    w1t_d = nc.dram_tensor("w1t", [HPC, HD, HDH], BF16, kind="ExternalInput")
    w1n_d = nc.dram_tensor("w1n", [HPC, HDH, HD], F32, kind="ExternalInput")
    w2t_d = nc.dram_tensor("w2t", [HPC, HDH, HD], BF16, kind="ExternalInput")
    w2nb_d = nc.dram_tensor("w2nb", [HPC, HD, HDH], BF16, kind="ExternalInput")
    w2n_d = nc.dram_tensor("w2n", [HPC, HD, HDH], F32, kind="ExternalInput")
    owT_d = nc.dram_tensor("owT", [HPC * HD, D], BF16, kind="ExternalInput")

    out_d = nc.dram_tensor("out_p", [L, D], F32, kind="ExternalOutput")
    w1next_d = nc.dram_tensor("w1next", [HPC, HDH, HD], F32, kind="ExternalOutput")
    w2next_d = nc.dram_tensor("w2next", [HPC, HD, HDH], F32, kind="ExternalOutput")

    triu_d = nc.inline_tensor(np.triu(np.ones((128, 128), np.float32)), "triu")
    ident_d = nc.inline_tensor(np.eye(128, dtype=np.float32), "ident")
    identb_d = nc.inline_tensor(np.eye(128, dtype=BFNP), "identb")
    ones_d = nc.inline_tensor(np.ones((128, 512), np.float32), "ones")

    with tile.TileContext(nc) as tc:
        with (
            tc.tile_pool(name="big", bufs=8) as big,       # 4KB/partition slots
            tc.tile_pool(name="bigq", bufs=8) as bigq,
            tc.tile_pool(name="qkv", bufs=1) as qkvp,
            tc.tile_pool(name="const", bufs=1) as cst,
            tc.tile_pool(name="scal", bufs=1) as scp,
            tc.tile_pool(name="head", bufs=8) as hp,
            tc.tile_pool(name="head2", bufs=2) as hp2,
            tc.tile_pool(name="stile", bufs=16) as sp,
            tc.tile_pool(name="tmp", bufs=2) as tmp,
            tc.tile_pool(name="psA", bufs=3, space="PSUM") as psA,
            tc.tile_pool(name="psT", bufs=2, space="PSUM") as psT,
            tc.tile_pool(name="psB", bufs=2, space="PSUM") as psB,
            tc.tile_pool(name="psW", bufs=1, space="PSUM") as psW,
        ):
            dma = nc.sync.dma_start

            # ---- constants & inputs to SBUF ----
            triu = cst.tile([128, 128], F32, tag="triu")
            ident = cst.tile([128, 128], F32, tag="ident")
            identb = cst.tile([128, 128], BF16, tag="identb")
            ones = cst.tile([128, 512], F32, tag="ones")
            dma(triu[:], triu_d[:])
            dma(ident[:], ident_d[:])
            dma(identb[:], identb_d[:])
            dma(ones[:], ones_d[:])

            xt, xtb, qk = [], [], []
            for kt in range(NLT):
                t = big.tile([128, L], F32, tag="big", name=f"xt{kt}")
                dma(t[:], xT_d[kt * 128:(kt + 1) * 128, :])
                xt.append(t)
                tb = bigq.tile([128, L], BF16, tag="xtb", name=f"xtb{kt}")
                dma(tb[:], xTb_d[kt * 128:(kt + 1) * 128, :])
                xtb.append(tb)
                tq = bigq.tile([128, 3 * HPC * HD], BF16, tag="qk",
                               name=f"qk{kt}")
                dma(tq[:], qkvT_d[kt * 128:(kt + 1) * 128, :])
                qk.append(tq)
            fcp = cst.tile([128, L], F32, tag="fcp")
            fcbp = cst.tile([1, 128], F32, tag="fcbp")
            basep = cst.tile([128, 1], F32, tag="basep")
            qkvbT = cst.tile([128, 6], F32, tag="qkvbT")
            dma(fcp[:], fcp_d[:])
            dma(fcbp[:], fcbp_d[:])
            dma(basep[:], basep_d[:])
            dma(qkvbT[:], qkvbT_d[:])
            owt = []
            for kt in range(2):
                t = bigq.tile([128, D], BF16, tag="owt", name=f"owt{kt}",
                              bufs=2)
                dma(t[:], owT_d[kt * 128:(kt + 1) * 128, :])
                owt.append(t)

            # ---- stage A: fc projections (T layout, head n mapped to
            # partition 32n (wd) / 32n+1 (lr)), log_wd, cumsum, scalars ----
            sigTp = scp.tile([128, L], F32, tag="sigTp")
            for ch in range(NCH):
                psfc = psA.tile([128, 512], F32, tag="psA")
                for kt in range(NLT):
                    nc.tensor.matmul(
                        psfc[:], fcp[:, kt * 128:(kt + 1) * 128],
                        xt[kt][:, ch * 512:(ch + 1) * 512],
                        start=(kt == 0), stop=False)
                nc.tensor.matmul(psfc[:], fcbp[:], ones[0:1, 0:512],
                                 start=False, stop=True)
                nc.scalar.activation(sigTp[:, ch * 512:(ch + 1) * 512],
                                     psfc[:], AF.Sigmoid)
            sbTp = scp.tile([128, L], F32, tag="sbTp")
            nc.vector.tensor_scalar_mul(sbTp[:], sigTp[:], basep[:])
            for n in range(HPC):
                nc.scalar.activation(sbTp[32 * n:32 * n + 1, :],
                                     sbTp[32 * n:32 * n + 1, :],
                                     AF.Ln, bias=1.0, scale=-1.0)
            # natural tiles: nat[lt][:, 32n] = log_wd, [:, 32n+1] = -lr*sig
            nat = []
            for lt in range(NLT):
                psn = psB.tile([128, 128], F32, tag="psB")
                nc.tensor.transpose(psn[:], sbTp[:, lt * 128:(lt + 1) * 128],
                                    ident[:])
                nt = scp.tile([128, 128], F32, tag=f"nat{lt}")
                nc.vector.tensor_copy(nt[:], psn[:])
                nat.append(nt)

            # cumT (T layout, heads at partitions 0/32/64/96)
            cumTp = scp.tile([128, L], F32, tag="cumTp")
            for mt in range(NLT):
                pscum = psB.tile([128, 128], F32, tag="psB")
                for lt in range(mt):
                    nc.tensor.matmul(pscum[:], nat[lt][:], ones[0:128, 0:128],
                                     start=(lt == 0), stop=False)
                nc.tensor.matmul(pscum[:], nat[mt][:], triu[:],
                                 start=(mt == 0), stop=True)
                nc.vector.tensor_copy(cumTp[:, mt * 128:(mt + 1) * 128], pscum[:])
            wdcTp = scp.tile([128, L], F32, tag="wdcTp")
            nc.scalar.activation(wdcTp[:], cumTp[:], AF.Exp)

            # wd_last broadcast [128, 4] (col n = wd_cross[L-1] of head n)
            wdlast = scp.tile([128, HPC], F32, tag="wdlast")
            for n in range(HPC):
                pswl = psB.tile([128, 1], F32, tag="psB")
                nc.tensor.matmul(pswl[:], ones[32 * n:32 * n + 1, 0:128],
                                 wdcTp[32 * n:32 * n + 1, L - 1:L],
                                 tile_position=(32 * n, 0))
                nc.vector.tensor_copy(wdlast[:, n:n + 1], pswl[:])

            # natural-layout lrw[lt][:, n] = -lr[l] * exp(-cum[l])
            lrw = []
            for lt in range(NLT):
                psct = psB.tile([128, 128], F32, tag="psB")
                nc.tensor.transpose(psct[:], cumTp[:, lt * 128:(lt + 1) * 128],
                                    ident[:])
                lw_t = scp.tile([128, HPC], F32, tag=f"lrw{lt}")
                for n in range(HPC):
                    iw = tmp.tile([128, 1], F32, tag="iw")
                    nc.scalar.activation(iw[:], psct[:, 32 * n:32 * n + 1],
                                         AF.Exp, scale=-1.0)
                    nc.vector.tensor_mul(lw_t[:, n:n + 1], iw[:],
                                         nat[lt][:, 32 * n + 1:32 * n + 2])
                lrw.append(lw_t)

            # ---- stage B: q/k/v projections in T layout (bf16) ----
            qkv_t = []
            for j in range(3):
                pair_tiles = []
                for mg in range(2):
                    dst = qkvp.tile([128, L], BF16, tag=f"qkv{j}{mg}")
                    off = j * HPC * HD + mg * 128
                    for ch in range(NCH):
                        psp = psA.tile([128, 512], F32, tag="psA")
                        for kt in range(NLT):
                            nc.tensor.matmul(
                                psp[:], qk[kt][:, off:off + 128],
                                xtb[kt][:, ch * 512:(ch + 1) * 512],
                                start=(kt == 0), stop=(kt == NLT - 1))
                        nc.vector.tensor_scalar_add(
                            dst[:, ch * 512:(ch + 1) * 512], psp[:],
                            qkvbT[:, j * 2 + mg:j * 2 + mg + 1])
                    pair_tiles.append(dst)
                qkv_t.append(pair_tiles)
            Q2T, K2T, V2T = qkv_t

            # Z2catT: pair tile kt holds heads 2kt, 2kt+1 (rows 0:64 / 64:128)
            z2cat = [big.tile([128, L], BF16, tag="big", name=f"z2cat{i}")
                     for i in range(2)]

            # ---- stage C/D: heads processed in pairs ----
            for pair in range(2):
                heads = [2 * pair, 2 * pair + 1]
                st_ = {}

                for n in heads:
                    bp = (n % 2) * 64
                    X1T = K2T[pair][bp:bp + 64, :]
                    VT = V2T[pair][bp:bp + 64, :]

                    w1t_s = hp2.tile([128, HDH], BF16, tag="w1t")
                    dma(w1t_s[0:64, :], w1t_d[n])
                    dma(w1t_s[64:128, :], w1t_d[n])
                    w1n_s = hp2.tile([HDH, HD], F32, tag="w1n")
                    dma(w1n_s[:], w1n_d[n])
                    w2t_s = hp2.tile([HDH, HD], BF16, tag="w2t")
                    dma(w2t_s[:], w2t_d[n])
                    w2nb_s = hp2.tile([HD, HDH], BF16, tag="w2nb")
                    dma(w2nb_s[:], w2nb_d[n])
                    w2n_s = hp2.tile([HD, HDH], F32, tag="w2n")
                    dma(w2n_s[:], w2n_d[n])

                    # forward, packed in 512-col halves (4 l-tiles each)
                    x2nh, dslh = [], []
                    X2T = big.tile([128, L], BF16, tag="big", name=f"X2T{n}")
                    for hf in range(2):
                        h0 = hf * 512
                        psZ = psA.tile([128, 512], F32, tag="psA")
                        for j in range(4):
                            c0 = h0 + j * 128
                            nc.tensor.matmul(psZ[:, j * 128:(j + 1) * 128],
                                             X1T[:, c0:c0 + 128],
                                             w1t_s[bp:bp + 64, :])
                        sg = tmp.tile([128, 512], F32, tag="sg")
                        nc.scalar.activation(sg[:], psZ[:], AF.Sigmoid)
                        z1s = tmp.tile([128, 512], F32, tag="z1s")
                        nc.vector.tensor_copy(z1s[:], psZ[:])
                        xa = hp.tile([128, 512], BF16, tag="x2n", bufs=8)
                        nc.vector.tensor_mul(xa[:], z1s[:], sg[:])   # silu
                        x2nh.append(xa)
                        # dsilu = sil + sg*(1 - sil)
                        w_ = tmp.tile([128, 512], F32, tag="w_")
                        nc.vector.tensor_scalar(w_[:], xa[:], -1.0, 1.0,
                                                mybir.AluOpType.mult,
                                                mybir.AluOpType.add)
                        da = hp.tile([128, 512], F32, tag="dsl", bufs=8)
                        nc.vector.scalar_tensor_tensor(
                            da[:], sg[:], 1.0, w_[:],
                            mybir.AluOpType.mult, mybir.AluOpType.mult)
                        nc.vector.tensor_add(da[:], da[:], xa[:])
                        dslh.append(da)
                        psX = psA.tile([128, 512], BF16, tag="psA")
                        for j in range(4):
                            nc.tensor.transpose(psX[:, j * 128:(j + 1) * 128],
                                                xa[:, j * 128:(j + 1) * 128],
                                                identb[:])
                        nc.vector.tensor_copy(X2T[:, h0:h0 + 512], psX[:])

                    gZ2T = hp2.tile([HD, L], BF16, tag="gz2t")
                    for ch in range(NCH):
                        ps3 = psA.tile([HD, 512], F32, tag="psA")
                        nc.tensor.matmul(ps3[:], w2t_s[:],
                                         X2T[:, ch * 512:(ch + 1) * 512])
                        nc.vector.tensor_sub(gZ2T[:, ch * 512:(ch + 1) * 512],
                                             ps3[:], VT[:, ch * 512:(ch + 1) * 512])

                    nG1h, nG2h, X1nh = [], [], []
                    for hf in range(2):
                        h0 = hf * 512
                        psG = psA.tile([128, 512], F32, tag="psA")
                        for j in range(4):
                            c0 = h0 + j * 128
                            nc.tensor.matmul(psG[:, j * 128:(j + 1) * 128],
                                             gZ2T[:, c0:c0 + 128], w2nb_s[:])
                        t1 = tmp.tile([128, 512], F32, tag="t1")
                        nc.vector.tensor_mul(t1[:], psG[:], dslh[hf][:])
                        g1 = hp.tile([128, 512], BF16, tag="ng1", bufs=8)
                        for j in range(4):
                            lt = hf * 4 + j
                            nc.vector.tensor_scalar_mul(
                                g1[:, j * 128:(j + 1) * 128],
                                t1[:, j * 128:(j + 1) * 128],
                                lrw[lt][:, n:n + 1])
                        nG1h.append(g1)
                        psU = psA.tile([128, 256], BF16, tag="psA",
                                       name="psU")
                        for j in range(4):
                            c0 = h0 + j * 128
                            nc.tensor.transpose(psU[:, j * 64:(j + 1) * 64],
                                                gZ2T[:, c0:c0 + 128],
                                                identb[0:64, 0:64])
                        g2 = hp.tile([128, 256], BF16, tag="ng2", bufs=8)
                        for j in range(4):
                            lt = hf * 4 + j
                            nc.vector.tensor_scalar_mul(
                                g2[:, j * 64:(j + 1) * 64],
                                psU[:, j * 64:(j + 1) * 64],
                                lrw[lt][:, n:n + 1])
                        nG2h.append(g2)
                        psV = psA.tile([128, 256], BF16, tag="psA",
                                       name="psV")
                        for j in range(4):
                            c0 = h0 + j * 128
                            nc.tensor.transpose(psV[:, j * 64:(j + 1) * 64],
                                                X1T[:, c0:c0 + 128],
                                                identb[bp:bp + 64, bp:bp + 64])
                        x1 = hp.tile([128, 256], BF16, tag="x1n", bufs=8)
                        nc.vector.tensor_copy(x1[:], psV[:])
                        X1nh.append(x1)

                    # weight updates
                    psw1 = psW.tile([HDH, HD], F32, tag="psW")
                    for lt in range(NLT):
                        hf, j = lt // 4, lt % 4
                        nc.tensor.matmul(psw1[:],
                                         nG1h[hf][:, j * 128:(j + 1) * 128],
                                         X1nh[hf][:, j * 64:(j + 1) * 64],
                                         start=(lt == 0), stop=(lt == NLT - 1))
                    tw1 = tmp.tile([HDH, HD], F32, tag="tw1")
                    nc.vector.tensor_add(tw1[:], psw1[:], w1n_s[:])
                    nc.vector.tensor_scalar_mul(tw1[:], tw1[:], wdlast[:, n:n + 1])
                    dma(w1next_d[n], tw1[:])
                    psw2 = psW.tile([HD, HDH], F32, tag="psW")
                    for lt in range(NLT):
                        hf, j = lt // 4, lt % 4
                        nc.tensor.matmul(psw2[:],
                                         nG2h[hf][:, j * 64:(j + 1) * 64],
                                         x2nh[hf][:, j * 128:(j + 1) * 128],
                                         start=(lt == 0), stop=(lt == NLT - 1))
                    tw2 = tmp.tile([HD, HDH], F32, tag="tw2")
                    nc.vector.tensor_add(tw2[:], psw2[:], w2n_s[0:64, :])
                    nc.vector.tensor_scalar_mul(tw2[:], tw2[:],
                                                wdlast[0:64, n:n + 1])
                    dma(w2next_d[n], tw2[:])

                    st_[n] = dict(
                        X2T=X2T, nG1h=nG1h, nG2h=nG2h, w1t_s=w1t_s, w2t_s=w2t_s,
                        X2_T=big.tile([128, L], BF16, tag="big", name=f"X2_T{n}"))

                # readout: interleave the two heads per 512-chunk
                for ch in range(NCH):
                    m0, m1 = ch * 512, (ch + 1) * 512
                    mb0 = 4 * ch
                    nlts = 4 * ch + 4

                    for n in heads:
                        s = st_[n]
                        bp = (n % 2) * 64
                        X1T = K2T[pair][bp:bp + 64, :]
                        X1_T = Q2T[pair][bp:bp + 64, :]

                        wdb = hp2.tile([128, 512], F32, tag="wdb", bufs=4)
                        pswb = psA.tile([128, 512], F32, tag="psA")
                        nc.tensor.matmul(pswb[:], ones[32 * n:32 * n + 1, 0:128],
                                         wdcTp[32 * n:32 * n + 1, m0:m1],
                                         tile_position=(32 * n, 0))
                        nc.vector.tensor_copy(wdb[:], pswb[:])
                        s["wdb"] = wdb

                        S = []
                        for lt in range(nlts):
                            j0 = 0 if lt < mb0 else (lt - mb0) * 128
                            psS = psA.tile([128, 512], F32, tag="psA")
                            nc.tensor.matmul(psS[:, j0:512],
                                             X1T[:, lt * 128:(lt + 1) * 128],
                                             X1_T[:, m0 + j0:m1])
                            st = sp.tile([128, 512], BF16, tag="s")
                            if lt < mb0:
                                if lt % 2 == 0:
                                    nc.scalar.copy(st[:], psS[:])
                                else:
                                    nc.vector.tensor_copy(st[:], psS[:])
                            else:
                                nc.vector.tensor_mul(st[:, j0:j0 + 128],
                                                     psS[:, j0:j0 + 128], triu[:])
                                if j0 + 128 < 512:
                                    nc.scalar.copy(st[:, j0 + 128:512],
                                                   psS[:, j0 + 128:512])
                            S.append(st)

                        psT1 = psT.tile([128, 512], F32, tag="psT")
                        for lt in range(nlts):
                            j0 = 0 if lt < mb0 else (lt - mb0) * 128
                            nc.tensor.matmul(psT1[:, j0:512],
                                             s["nG1h"][lt // 4]
                                             [:, (lt % 4) * 128:(lt % 4 + 1) * 128],
                                             S[lt][:, j0:512],
                                             start=(lt == 0), stop=False)
                        nc.tensor.matmul(psT1[:], s["w1t_s"][bp:bp + 64, :],
                                         X1_T[:, m0:m1], start=False, stop=True)
                        z1t = hp2.tile([128, 512], F32, tag="z1t", bufs=4)
                        nc.vector.tensor_mul(z1t[:], psT1[:], s["wdb"][:])
                        sgro = tmp.tile([128, 512], F32, tag="sgro")
                        nc.scalar.activation(sgro[:], z1t[:], AF.Sigmoid)
                        nc.vector.tensor_mul(s["X2_T"][:, m0:m1], z1t[:], sgro[:])

                    for n in heads:
                        s = st_[n]
                        bp = (n % 2) * 64

                        S2 = []
                        for lt in range(nlts):
                            j0 = 0 if lt < mb0 else (lt - mb0) * 128
                            psS2 = psA.tile([128, 512], F32, tag="psA")
                            nc.tensor.matmul(psS2[:, j0:512],
                                             s["X2T"][:, lt * 128:(lt + 1) * 128],
                                             s["X2_T"][:, m0 + j0:m1])
                            st = sp.tile([128, 512], BF16, tag="s", name="s2t")
                            if lt < mb0:
                                if lt % 2 == 0:
                                    nc.vector.tensor_copy(st[:], psS2[:])
                                else:
                                    nc.scalar.copy(st[:], psS2[:])
                            else:
                                nc.vector.tensor_mul(st[:, j0:j0 + 128],
                                                     psS2[:, j0:j0 + 128],
                                                     triu[:])
                                if j0 + 128 < 512:
                                    nc.scalar.copy(st[:, j0 + 128:512],
                                                   psS2[:, j0 + 128:512])
                            S2.append(st)

                        psT2 = psT.tile([HD, 512], F32, tag="psT")
                        for lt in range(nlts):
                            j0 = 0 if lt < mb0 else (lt - mb0) * 128
                            nc.tensor.matmul(psT2[:, j0:512],
                                             s["nG2h"][lt // 4]
                                             [:, (lt % 4) * 64:(lt % 4 + 1) * 64],
                                             S2[lt][:, j0:512],
                                             start=(lt == 0), stop=False)
                        nc.tensor.matmul(psT2[:], s["w2t_s"][:],
                                         s["X2_T"][:, m0:m1],
                                         start=False, stop=True)
                        nc.vector.tensor_mul(z2cat[pair][bp:bp + 64, m0:m1],
                                             psT2[:], s["wdb"][0:64, :])

            # ---- stage E: o-projection (partial, heads of this core) ----
            for lt in range(NLT):
                outs = big.tile([128, D], F32, tag="big", name=f"outs{lt}")
                for ch in range(NCH):
                    psO = psA.tile([128, 512], F32, tag="psA")
                    for kt in range(2):
                        nc.tensor.matmul(
                            psO[:], z2cat[kt][:, lt * 128:(lt + 1) * 128],
                            owt[kt][:, ch * 512:(ch + 1) * 512],
                            start=(kt == 0), stop=(kt == 1))
                    nc.vector.tensor_copy(outs[:, ch * 512:(ch + 1) * 512],
                                          psO[:])
                dma(out_d[lt * 128:(lt + 1) * 128, :], outs[:])

    nc.compile()
    return nc


_NC = None


def _get_nc():
    global _NC
    if _NC is None:
        _NC = build_program()
    return _NC


def make_in_maps(x, W1, W2, log_base_lr, fc_lr_w, fc_lr_b, log_base_weight_decay,
                 fc_wd_w, fc_wd_b, q_w, q_b, k_w, k_b, v_w, v_b, o_w, o_b):
    f = np.float32
    in_maps = []
    for core in range(NCORES):
        b, g = core // 4, core % 4
        hs = slice(g * HPC, (g + 1) * HPC)
        cs = slice(g * HPC * HD, (g + 1) * HPC * HD)
        xT = np.ascontiguousarray(x[b].T, dtype=f)
        qkvT = np.ascontiguousarray(
            np.concatenate([q_w[cs, :].T, k_w[cs, :].T, v_w[cs, :].T], axis=1),
            dtype=BFNP)
        # fc weights padded: within each 128-col k-block, col 32n = wd head n,
        # col 32n+1 = lr head n (others zero)
        fcp = np.zeros((D, 128), f)
        fcbp = np.zeros((1, 128), f)
        basep = np.zeros((128, 1), f)
        for n in range(HPC):
            fcp[:, 32 * n] = fc_wd_w[g * HPC + n, :]
            fcp[:, 32 * n + 1] = fc_lr_w[g * HPC + n, :]
            fcbp[0, 32 * n] = fc_wd_b[g * HPC + n]
            fcbp[0, 32 * n + 1] = fc_lr_b[g * HPC + n]
            basep[32 * n, 0] = np.exp(log_base_weight_decay[g * HPC + n])
            basep[32 * n + 1, 0] = -np.exp(log_base_lr[g * HPC + n])
        fcp = np.ascontiguousarray(
            fcp.reshape(8, 128, 128).transpose(1, 0, 2).reshape(128, 1024))
        qkvbT = np.zeros((128, 6), f)
        for j, bias in enumerate([q_b, k_b, v_b]):
            for mg in range(2):
                qkvbT[:, j * 2 + mg] = bias[cs][mg * 128:(mg + 1) * 128]
        w1 = W1[b, hs]
        w2 = W2[b, hs]
        w1t = np.ascontiguousarray(w1.transpose(0, 2, 1), dtype=BFNP)
        w2t = np.ascontiguousarray(w2.transpose(0, 2, 1), dtype=BFNP)
        owT = np.ascontiguousarray(o_w[:, cs].T, dtype=BFNP)
        in_maps.append({
            "xT": xT, "xTb": xT.astype(BFNP), "qkvT": qkvT,
            "fcp": fcp, "fcbp": fcbp, "basep": basep, "qkvbT": qkvbT,
            "w1t": w1t, "w1n": np.ascontiguousarray(w1, dtype=f),
            "w2t": w2t, "w2nb": np.ascontiguousarray(w2, dtype=BFNP),
            "w2n": np.ascontiguousarray(w2, dtype=f),
            "owT": owT,
        })
    return in_maps


def run(inputs, trace=False):
    nc = _get_nc()
    in_maps = make_in_maps(**inputs)
    res = run_bass_kernel_spmd(nc, in_maps, list(range(NCORES)), trace=trace)
    out = np.zeros((B, L, D), np.float32)
    W1n = np.zeros((B, NH, HDH, HD), np.float32)
    W2n = np.zeros((B, NH, HD, HDH), np.float32)
    for core in range(NCORES):
        b, g = core // 4, core % 4
        r = res.results[core]
        out[b] += r["out_p"]
        W1n[b, g * HPC:(g + 1) * HPC] = r["w1next"]
        W2n[b, g * HPC:(g + 1) * HPC] = r["w2next"]
    out += np.asarray(inputs["o_b"], np.float32)[None, None, :]
    return (out, W1n, W2n), res


def kernel(**inputs):
    inputs = {k: np.asarray(v) for k, v in inputs.items()}
    (out, W1n, W2n), _ = run(inputs)
    return out, W1n, W2n


if __name__ == "__main__":
    print("building program...")
    nc = _get_nc()
    print("built ok")


# revision 34
# speedup vs baseline: 2.9575x; 1.2323x over previous
"""Trainium2 Bass kernel for nn_MultiHeadNeuralMemoryMLP.

Math reformulation (per batch b, head n), avoiding the [L, L] decay masks:
  cum[l]      = cumsum(log_wd)[l],  wd_cross[l] = exp(cum[l])
  wd_inner[l, m] = wd_cross[m] / wd_cross[l]  (for l <= m)
  nG1 = -grad_Z1 * (lr * exp(-cum))[:, None]   (lr sign folded in)
  Z1_[m] = wd_cross[m] * (S_masked.T @ nG1 + X1_ @ W1.T)[m],  S = X1 @ X1_.T causal
  Z2_[m] = wd_cross[m] * (S2_masked.T @ nG2 + X2_ @ W2.T)[m], S2 = X2 @ X2_.T causal
  W1_next = wd_cross[L-1] * (W1 + nG1.T @ X1);  W2_next similarly.

Sharding: core = b * 4 + g handles batch b, heads 4g..4g+3. Projections use
replicated (sliced) weights; only the o-projection needs a cross-core sum,
done on the host over 4 partial [L, D] tensors per batch.

Matmul operands are bf16 (f32 PSUM accumulation); the scalar pipeline
(fc projections, log-weight-decay cumsum, per-token scalars) stays f32.
The scalar engine runs only Sigmoid in steady state (silu/dsilu composed
on DVE) to avoid activation-table reloads.
"""

import sys

import numpy as np
import ml_dtypes

if "/opt/trn_rl_repo" not in sys.path:
    sys.path.insert(0, "/opt/trn_rl_repo")

import concourse.bass as bass
import concourse.mybir as mybir
import concourse.tile as tile
from concourse import bacc
from concourse.bass_utils import run_bass_kernel_spmd

F32 = mybir.dt.float32
BF16 = mybir.dt.bfloat16
AF = mybir.ActivationFunctionType
BFNP = ml_dtypes.bfloat16

B, L, D, NH, DH = 2, 1024, 1024, 16, 2048
HD, HDH = D // NH, DH // NH          # 64, 128
HPC = 4                               # heads per core
NCORES = 8
NLT = L // 128                        # 8 row tiles
NCH = L // 512                        # 2 column chunks


def build_program():
    nc = bacc.Bacc("TRN2", target_bir_lowering=False, debug=False,
                   num_devices=NCORES)

    # ---- DRAM I/O ----
    xTb_d = nc.dram_tensor("xTb", [D, L], BF16, kind="ExternalInput")
    qkvT_d = nc.dram_tensor("qkvT", [D, 3 * HPC * HD], BF16, kind="ExternalInput")
    fcp_d = nc.dram_tensor("fcp", [128, L], BF16, kind="ExternalInput")
    fcbp_d = nc.dram_tensor("fcbp", [1, 128], F32, kind="ExternalInput")
    basep_d = nc.dram_tensor("basep", [128, 1], F32, kind="ExternalInput")
    qkvbT_d = nc.dram_tensor("qkvbT", [128, 6], F32, kind="ExternalInput")
    w1t_d = nc.dram_tensor("w1t", [HPC, HD, HDH], BF16, kind="ExternalInput")
    w1n_d = nc.dram_tensor("w1n", [HPC, HDH, HD], F32, kind="ExternalInput")
    w2t_d = nc.dram_tensor("w2t", [HPC, HDH, HD], BF16, kind="ExternalInput")
    w2nb_d = nc.dram_tensor("w2nb", [HPC, HD, HDH], BF16, kind="ExternalInput")
    w2n_d = nc.dram_tensor("w2n", [HPC, HD, HDH], F32, kind="ExternalInput")
    owT_d = nc.dram_tensor("owT", [HPC * HD, D], BF16, kind="ExternalInput")

    out_d = nc.dram_tensor("out_p", [L, D], F32, kind="ExternalOutput")
    w1next_d = nc.dram_tensor("w1next", [HPC, HDH, HD], F32, kind="ExternalOutput")
    w2next_d = nc.dram_tensor("w2next", [HPC, HD, HDH], F32, kind="ExternalOutput")

    triu_d = nc.inline_tensor(np.triu(np.ones((128, 128), np.float32)), "triu")
    ident_d = nc.inline_tensor(np.eye(128, dtype=np.float32), "ident")
    identb_d = nc.inline_tensor(np.eye(128, dtype=BFNP), "identb")
    ones_d = nc.inline_tensor(np.ones((128, 512), np.float32), "ones")

    with tile.TileContext(nc) as tc:
        with (
            tc.tile_pool(name="big", bufs=8) as big,       # 4KB/partition slots
            tc.tile_pool(name="bigq", bufs=8) as bigq,
            tc.tile_pool(name="qkv", bufs=1) as qkvp,
            tc.tile_pool(name="const", bufs=1) as cst,
            tc.tile_pool(name="scal", bufs=1) as scp,
            tc.tile_pool(name="head", bufs=8) as hp,
            tc.tile_pool(name="head2", bufs=2) as hp2,
            tc.tile_pool(name="stile", bufs=16) as sp,
            tc.tile_pool(name="tmp", bufs=2) as tmp,
            tc.tile_pool(name="psA", bufs=3, space="PSUM") as psA,
            tc.tile_pool(name="psT", bufs=2, space="PSUM") as psT,
            tc.tile_pool(name="psB", bufs=2, space="PSUM") as psB,
            tc.tile_pool(name="psW", bufs=1, space="PSUM") as psW,
        ):
            dma = nc.sync.dma_start

            # ---- constants & inputs to SBUF ----
            triu = cst.tile([128, 128], F32, tag="triu")
            ident = cst.tile([128, 128], F32, tag="ident")
            identb = cst.tile([128, 128], BF16, tag="identb")
            ones = cst.tile([128, 512], F32, tag="ones")
            dma(triu[:], triu_d[:])
            dma(ident[:], ident_d[:])
            dma(identb[:], identb_d[:])
            dma(ones[:], ones_d[:])

            xtb, qk = [], []
            for kt in range(NLT):
                tb = bigq.tile([128, L], BF16, tag="xtb", name=f"xtb{kt}")
                dma(tb[:], xTb_d[kt * 128:(kt + 1) * 128, :])
                xtb.append(tb)
                tq = bigq.tile([128, 3 * HPC * HD], BF16, tag="qk",
                               name=f"qk{kt}")
                dma(tq[:], qkvT_d[kt * 128:(kt + 1) * 128, :])
                qk.append(tq)
            fcp = cst.tile([128, L], BF16, tag="fcp")
            fcbp = cst.tile([1, 128], F32, tag="fcbp")
            basep = cst.tile([128, 1], F32, tag="basep")
            qkvbT = cst.tile([128, 6], F32, tag="qkvbT")
            dma(fcp[:], fcp_d[:])
            dma(fcbp[:], fcbp_d[:])
            dma(basep[:], basep_d[:])
            dma(qkvbT[:], qkvbT_d[:])
            owt = []
            for kt in range(2):
                t = bigq.tile([128, D], BF16, tag="owt", name=f"owt{kt}",
                              bufs=2)
                dma(t[:], owT_d[kt * 128:(kt + 1) * 128, :])
                owt.append(t)

            # ---- stage A: fc projections (T layout, head n mapped to
            # partition 32n (wd) / 32n+1 (lr)), log_wd, cumsum, scalars ----
            sigTp = scp.tile([128, L], F32, tag="sigTp")
            for ch in range(NCH):
                psfc = psA.tile([128, 512], F32, tag="psA")
                for kt in range(NLT):
                    nc.tensor.matmul(
                        psfc[:], fcp[:, kt * 128:(kt + 1) * 128],
                        xtb[kt][:, ch * 512:(ch + 1) * 512],
                        start=(kt == 0), stop=False)
                nc.tensor.matmul(psfc[:], fcbp[:], ones[0:1, 0:512],
                                 start=False, stop=True)
                nc.scalar.activation(sigTp[:, ch * 512:(ch + 1) * 512],
                                     psfc[:], AF.Sigmoid)
            sbTp = scp.tile([128, L], F32, tag="sbTp")
            nc.vector.tensor_scalar_mul(sbTp[:], sigTp[:], basep[:])
            for n in range(HPC):
                nc.scalar.activation(sbTp[32 * n:32 * n + 1, :],
                                     sbTp[32 * n:32 * n + 1, :],
                                     AF.Ln, bias=1.0, scale=-1.0)
            # natural tiles: nat[lt][:, 32n] = log_wd, [:, 32n+1] = -lr*sig
            nat = []
            for lt in range(NLT):
                psn = psB.tile([128, 128], F32, tag="psB")
                nc.tensor.transpose(psn[:], sbTp[:, lt * 128:(lt + 1) * 128],
                                    ident[:])
                nt = scp.tile([128, 128], F32, tag=f"nat{lt}")
                nc.scalar.copy(nt[:], psn[:])
                nat.append(nt)

            # cumT (T layout, heads at partitions 0/32/64/96)
            cumTp = scp.tile([128, L], F32, tag="cumTp")
            for mt in range(NLT):
                pscum = psB.tile([128, 128], F32, tag="psB")
                for lt in range(mt):
                    nc.tensor.matmul(pscum[:], nat[lt][:], ones[0:128, 0:128],
                                     start=(lt == 0), stop=False)
                nc.tensor.matmul(pscum[:], nat[mt][:], triu[:],
                                 start=(mt == 0), stop=True)
                nc.scalar.copy(cumTp[:, mt * 128:(mt + 1) * 128], pscum[:])
            wdcTp = scp.tile([128, L], F32, tag="wdcTp")
            nc.scalar.activation(wdcTp[:], cumTp[:], AF.Exp)

            # wd_last broadcast [128, 4] (col n = wd_cross[L-1] of head n)
            wdlast = scp.tile([128, HPC], F32, tag="wdlast")
            for n in range(HPC):
                pswl = psB.tile([128, 1], F32, tag="psB")
                nc.tensor.matmul(pswl[:], ones[32 * n:32 * n + 1, 0:128],
                                 wdcTp[32 * n:32 * n + 1, L - 1:L],
                                 tile_position=(32 * n, 0))
                nc.vector.tensor_copy(wdlast[:, n:n + 1], pswl[:])

            # natural-layout lrw[lt][:, n] = -lr[l] * exp(-cum[l])
            lrw = []
            for lt in range(NLT):
                psct = psB.tile([128, 128], F32, tag="psB")
                nc.tensor.transpose(psct[:], cumTp[:, lt * 128:(lt + 1) * 128],
                                    ident[:])
                lw_t = scp.tile([128, HPC], F32, tag=f"lrw{lt}")
                for n in range(HPC):
                    iw = tmp.tile([128, 1], F32, tag="iw")
                    nc.scalar.activation(iw[:], psct[:, 32 * n:32 * n + 1],
                                         AF.Exp, scale=-1.0)
                    nc.vector.tensor_mul(lw_t[:, n:n + 1], iw[:],
                                         nat[lt][:, 32 * n + 1:32 * n + 2])
                lrw.append(lw_t)

            # ---- stage B: q/k/v projections in T layout (bf16) ----
            qkv_t = []
            for j in range(3):
                pair_tiles = []
                for mg in range(2):
                    dst = qkvp.tile([128, L], BF16, tag=f"qkv{j}{mg}")
                    off = j * HPC * HD + mg * 128
                    for ch in range(NCH):
                        psp = psA.tile([128, 512], F32, tag="psA")
                        for kt in range(NLT):
                            nc.tensor.matmul(
                                psp[:], qk[kt][:, off:off + 128],
                                xtb[kt][:, ch * 512:(ch + 1) * 512],
                                start=(kt == 0), stop=(kt == NLT - 1))
                        nc.vector.tensor_scalar_add(
                            dst[:, ch * 512:(ch + 1) * 512], psp[:],
                            qkvbT[:, j * 2 + mg:j * 2 + mg + 1])
                    pair_tiles.append(dst)
                qkv_t.append(pair_tiles)
            Q2T, K2T, V2T = qkv_t

            # Z2catT: pair tile kt holds heads 2kt, 2kt+1 (rows 0:64 / 64:128)
            z2cat = [big.tile([128, L], BF16, tag="big", name=f"z2cat{i}")
                     for i in range(2)]

            # wdb broadcast tiles ([128, 512] rows = wd_cross), built upfront
            wdbs = {}
            for n in range(HPC):
                for ch in range(NCH):
                    m0, m1 = ch * 512, (ch + 1) * 512
                    wdb = hp2.tile([128, 512], F32, tag="wdb", bufs=8,
                                   name=f"wdb{n}{ch}")
                    pswb = psA.tile([128, 512], F32, tag="psA")
                    nc.tensor.matmul(pswb[:], ones[32 * n:32 * n + 1, 0:128],
                                     wdcTp[32 * n:32 * n + 1, m0:m1],
                                     tile_position=(32 * n, 0))
                    nc.vector.tensor_copy(wdb[:], pswb[:])
                    wdbs[(n, ch)] = wdb

            # ---- stage C/D: heads processed in pairs, stage-major so the
            # static schedule interleaves the two heads' chains ----
            for pair in range(2):
                heads = [2 * pair, 2 * pair + 1]
                st_ = {}
                for n in heads:
                    bp = (n % 2) * 64
                    w1t_s = hp2.tile([128, HDH], BF16, tag="w1t",
                                     name=f"w1t{n}")
                    dma(w1t_s[0:64, :], w1t_d[n])
                    dma(w1t_s[64:128, :], w1t_d[n])
                    w1n_s = hp2.tile([HDH, HD], F32, tag="w1n", name=f"w1n{n}")
                    dma(w1n_s[:], w1n_d[n])
                    w2t_s = hp2.tile([HDH, HD], BF16, tag="w2t", name=f"w2t{n}")
                    dma(w2t_s[:], w2t_d[n])
                    w2nb_s = hp2.tile([HD, HDH], BF16, tag="w2nb",
                                      name=f"w2nb{n}")
                    dma(w2nb_s[:], w2nb_d[n])
                    w2n_s = hp2.tile([HD, HDH], F32, tag="w2n", name=f"w2n{n}")
                    dma(w2n_s[:], w2n_d[n])
                    st_[n] = dict(
                        bp=bp, w1t_s=w1t_s, w1n_s=w1n_s, w2t_s=w2t_s,
                        w2nb_s=w2nb_s, w2n_s=w2n_s,
                        X1T=K2T[pair][bp:bp + 64, :],
                        X1_T=Q2T[pair][bp:bp + 64, :],
                        VT=V2T[pair][bp:bp + 64, :],
                        X2T=big.tile([128, L], BF16, tag="big", name=f"X2T{n}"),
                        X2_T=big.tile([128, L], BF16, tag="big",
                                      name=f"X2_T{n}"),
                        gZ2T=hp2.tile([HD, L], BF16, tag="gz2t",
                                      name=f"gz2t{n}"),
                        x2nh=[], dslh=[], nG1h=[], nG2h=[], X1nh=[])

                # fwd stage 1: Z1 -> sigmoid -> silu/dsilu (per half, per head)
                for hf in range(2):
                    for n in heads:
                        s = st_[n]
                        bp, h0 = s["bp"], hf * 512
                        psZ = psA.tile([128, 512], F32, tag="psA")
                        for j in range(4):
                            c0 = h0 + j * 128
                            nc.tensor.matmul(psZ[:, j * 128:(j + 1) * 128],
                                             s["X1T"][:, c0:c0 + 128],
                                             s["w1t_s"][bp:bp + 64, :])
                        sg = tmp.tile([128, 512], F32, tag="sg")
                        nc.scalar.activation(sg[:], psZ[:], AF.Sigmoid)
                        z1s = tmp.tile([128, 512], F32, tag="z1s")
                        nc.vector.tensor_copy(z1s[:], psZ[:])
                        xa = hp.tile([128, 512], BF16, tag="x2n", bufs=8)
                        nc.vector.tensor_mul(xa[:], z1s[:], sg[:])   # silu
                        s["x2nh"].append(xa)
                        # dsilu = sil + sg*(1 - sil)
                        w_ = tmp.tile([128, 512], F32, tag="w_")
                        nc.vector.tensor_scalar(w_[:], xa[:], -1.0, 1.0,
                                                mybir.AluOpType.mult,
                                                mybir.AluOpType.add)
                        da = hp.tile([128, 512], F32, tag="dsl", bufs=4)
                        nc.vector.scalar_tensor_tensor(
                            da[:], sg[:], 1.0, w_[:],
                            mybir.AluOpType.mult, mybir.AluOpType.mult)
                        nc.vector.tensor_add(da[:], da[:], xa[:])
                        s["dslh"].append(da)

                # fwd stage 2: X2T transposes
                for hf in range(2):
                    for n in heads:
                        s = st_[n]
                        h0 = hf * 512
                        psX = psA.tile([128, 512], BF16, tag="psA", name="psX")
                        for j in range(4):
                            nc.tensor.transpose(
                                psX[:, j * 128:(j + 1) * 128],
                                s["x2nh"][hf][:, j * 128:(j + 1) * 128],
                                identb[:])
                        nc.scalar.copy(s["X2T"][:, h0:h0 + 512], psX[:])

                # fwd stage 3: Z2T + grad_Z2
                for n in heads:
                    s = st_[n]
                    for ch in range(NCH):
                        ps3 = psA.tile([HD, 512], F32, tag="psA")
                        nc.tensor.matmul(ps3[:], s["w2t_s"][:],
                                         s["X2T"][:, ch * 512:(ch + 1) * 512])
                        nc.vector.tensor_sub(
                            s["gZ2T"][:, ch * 512:(ch + 1) * 512], ps3[:],
                            s["VT"][:, ch * 512:(ch + 1) * 512])

                # fwd stage 4: grad_X2, nG1/nG2, X1 natural
                for hf in range(2):
                    for n in heads:
                        s = st_[n]
                        bp, h0 = s["bp"], hf * 512
                        psG = psA.tile([128, 512], F32, tag="psA")
                        for j in range(4):
                            c0 = h0 + j * 128
                            nc.tensor.matmul(psG[:, j * 128:(j + 1) * 128],
                                             s["gZ2T"][:, c0:c0 + 128],
                                             s["w2nb_s"][:])
                        t1 = tmp.tile([128, 512], F32, tag="t1")
                        nc.vector.tensor_mul(t1[:], psG[:], s["dslh"][hf][:])
                        g1 = hp.tile([128, 512], BF16, tag="ng1", bufs=8)
                        for j in range(4):
                            lt = hf * 4 + j
                            nc.vector.tensor_scalar_mul(
                                g1[:, j * 128:(j + 1) * 128],
                                t1[:, j * 128:(j + 1) * 128],
                                lrw[lt][:, n:n + 1])
                        s["nG1h"].append(g1)
                        psU = psA.tile([128, 256], BF16, tag="psA", name="psU")
                        for j in range(4):
                            c0 = h0 + j * 128
                            nc.tensor.transpose(psU[:, j * 64:(j + 1) * 64],
                                                s["gZ2T"][:, c0:c0 + 128],
                                                identb[0:64, 0:64])
                        g2 = hp.tile([128, 256], BF16, tag="ng2", bufs=8)
                        for j in range(4):
                            lt = hf * 4 + j
                            nc.vector.tensor_scalar_mul(
                                g2[:, j * 64:(j + 1) * 64],
                                psU[:, j * 64:(j + 1) * 64],
                                lrw[lt][:, n:n + 1])
                        s["nG2h"].append(g2)
                        psV = psA.tile([128, 256], BF16, tag="psA", name="psV")
                        for j in range(4):
                            c0 = h0 + j * 128
                            nc.tensor.transpose(
                                psV[:, j * 64:(j + 1) * 64],
                                s["X1T"][:, c0:c0 + 128],
                                identb[bp:bp + 64, bp:bp + 64])
                        x1 = hp.tile([128, 256], BF16, tag="x1n", bufs=8)
                        nc.scalar.copy(x1[:], psV[:])
                        s["X1nh"].append(x1)

                # readout: interleave the two heads per 512-chunk
                for ch in range(NCH):
                    m0, m1 = ch * 512, (ch + 1) * 512
                    mb0 = 4 * ch
                    nlts = 4 * ch + 4

                    for n in heads:
                        s = st_[n]
                        bp = (n % 2) * 64
                        X1T = K2T[pair][bp:bp + 64, :]
                        X1_T = Q2T[pair][bp:bp + 64, :]

                        s["wdb"] = wdbs[(n, ch)]

                        S = []
                        for lt in range(nlts):
                            j0 = 0 if lt < mb0 else (lt - mb0) * 128
                            psS = psA.tile([128, 512], F32, tag="psA")
                            nc.tensor.matmul(psS[:, j0:512],
                                             X1T[:, lt * 128:(lt + 1) * 128],
                                             X1_T[:, m0 + j0:m1])
                            st = sp.tile([128, 512], BF16, tag="s")
                            if lt < mb0:
                                if lt % 2 == 0:
                                    nc.scalar.copy(st[:], psS[:])
                                else:
                                    nc.vector.tensor_copy(st[:], psS[:])
                            else:
                                nc.vector.tensor_mul(st[:, j0:j0 + 128],
                                                     psS[:, j0:j0 + 128], triu[:])
                                if j0 + 128 < 512:
                                    nc.scalar.copy(st[:, j0 + 128:512],
                                                   psS[:, j0 + 128:512])
                            S.append(st)

                        psT1 = psT.tile([128, 512], F32, tag="psT")
                        for lt in range(nlts):
                            j0 = 0 if lt < mb0 else (lt - mb0) * 128
                            nc.tensor.matmul(psT1[:, j0:512],
                                             s["nG1h"][lt // 4]
                                             [:, (lt % 4) * 128:(lt % 4 + 1) * 128],
                                             S[lt][:, j0:512],
                                             start=(lt == 0), stop=False)
                        nc.tensor.matmul(psT1[:], s["w1t_s"][bp:bp + 64, :],
                                         X1_T[:, m0:m1], start=False, stop=True)
                        z1t = hp2.tile([128, 512], F32, tag="z1t", bufs=4)
                        nc.vector.tensor_mul(z1t[:], psT1[:], s["wdb"][:])
                        sgro = tmp.tile([128, 512], F32, tag="sgro")
                        nc.scalar.activation(sgro[:], z1t[:], AF.Sigmoid)
                        nc.vector.tensor_mul(s["X2_T"][:, m0:m1], z1t[:], sgro[:])

                    for n in heads:
                        s = st_[n]
                        bp = (n % 2) * 64

                        S2 = []
                        for lt in range(nlts):
                            j0 = 0 if lt < mb0 else (lt - mb0) * 128
                            psS2 = psA.tile([128, 512], F32, tag="psA")
                            nc.tensor.matmul(psS2[:, j0:512],
                                             s["X2T"][:, lt * 128:(lt + 1) * 128],
                                             s["X2_T"][:, m0 + j0:m1])
                            st = sp.tile([128, 512], BF16, tag="s", name="s2t")
                            if lt < mb0:
                                if lt % 2 == 0:
                                    nc.vector.tensor_copy(st[:], psS2[:])
                                else:
                                    nc.scalar.copy(st[:], psS2[:])
                            else:
                                nc.vector.tensor_mul(st[:, j0:j0 + 128],
                                                     psS2[:, j0:j0 + 128],
                                                     triu[:])
                                if j0 + 128 < 512:
                                    nc.scalar.copy(st[:, j0 + 128:512],
                                                   psS2[:, j0 + 128:512])
                            S2.append(st)

                        psT2 = psT.tile([HD, 512], F32, tag="psT")
                        for lt in range(nlts):
                            j0 = 0 if lt < mb0 else (lt - mb0) * 128
                            nc.tensor.matmul(psT2[:, j0:512],
                                             s["nG2h"][lt // 4]
                                             [:, (lt % 4) * 64:(lt % 4 + 1) * 64],
                                             S2[lt][:, j0:512],
                                             start=(lt == 0), stop=False)
                        nc.tensor.matmul(psT2[:], s["w2t_s"][:],
                                         s["X2_T"][:, m0:m1],
                                         start=False, stop=True)
                        nc.vector.tensor_mul(z2cat[pair][bp:bp + 64, m0:m1],
                                             psT2[:], s["wdb"][0:64, :])

                # weight updates (off the readout critical path)
                for n in heads:
                    s = st_[n]
                    psw1 = psW.tile([HDH, HD], F32, tag="psW")
                    for lt in range(NLT):
                        hf, j = lt // 4, lt % 4
                        nc.tensor.matmul(psw1[:],
                                         s["nG1h"][hf][:, j * 128:(j + 1) * 128],
                                         s["X1nh"][hf][:, j * 64:(j + 1) * 64],
                                         start=(lt == 0), stop=(lt == NLT - 1))
                    tw1 = tmp.tile([HDH, HD], F32, tag="tw1")
                    nc.vector.tensor_add(tw1[:], psw1[:], s["w1n_s"][:])
                    nc.vector.tensor_scalar_mul(tw1[:], tw1[:],
                                                wdlast[:, n:n + 1])
                    dma(w1next_d[n], tw1[:])
                    psw2 = psW.tile([HD, HDH], F32, tag="psW")
                    for lt in range(NLT):
                        hf, j = lt // 4, lt % 4
                        nc.tensor.matmul(psw2[:],
                                         s["nG2h"][hf][:, j * 64:(j + 1) * 64],
                                         s["x2nh"][hf][:, j * 128:(j + 1) * 128],
                                         start=(lt == 0), stop=(lt == NLT - 1))
                    tw2 = tmp.tile([HD, HDH], F32, tag="tw2")
                    nc.vector.tensor_add(tw2[:], psw2[:], s["w2n_s"][0:64, :])
                    nc.vector.tensor_scalar_mul(tw2[:], tw2[:],
                                                wdlast[0:64, n:n + 1])
                    dma(w2next_d[n], tw2[:])

            # ---- stage E: o-projection (partial, heads of this core) ----
            for lt in range(NLT):
                outs = big.tile([128, D], F32, tag="big", name=f"outs{lt}")
                for ch in range(NCH):
                    psO = psA.tile([128, 512], F32, tag="psA")
                    for kt in range(2):
                        nc.tensor.matmul(
                            psO[:], z2cat[kt][:, lt * 128:(lt + 1) * 128],
                            owt[kt][:, ch * 512:(ch + 1) * 512],
                            start=(kt == 0), stop=(kt == 1))
                    nc.vector.tensor_copy(outs[:, ch * 512:(ch + 1) * 512],
                                          psO[:])
                dma(out_d[lt * 128:(lt + 1) * 128, :], outs[:])

    nc.compile()
    return nc


_NC = None


def _get_nc():
    global _NC
    if _NC is None:
        _NC = build_program()
    return _NC


def make_in_maps(x, W1, W2, log_base_lr, fc_lr_w, fc_lr_b, log_base_weight_decay,
                 fc_wd_w, fc_wd_b, q_w, q_b, k_w, k_b, v_w, v_b, o_w, o_b):
    f = np.float32
    in_maps = []
    for core in range(NCORES):
        b, g = core // 4, core % 4
        hs = slice(g * HPC, (g + 1) * HPC)
        cs = slice(g * HPC * HD, (g + 1) * HPC * HD)
        xT = np.ascontiguousarray(x[b].T, dtype=f)
        qkvT = np.ascontiguousarray(
            np.concatenate([q_w[cs, :].T, k_w[cs, :].T, v_w[cs, :].T], axis=1),
            dtype=BFNP)
        # fc weights padded: within each 128-col k-block, col 32n = wd head n,
        # col 32n+1 = lr head n (others zero)
        fcp = np.zeros((D, 128), f)
        fcbp = np.zeros((1, 128), f)
        basep = np.zeros((128, 1), f)
        for n in range(HPC):
            fcp[:, 32 * n] = fc_wd_w[g * HPC + n, :]
            fcp[:, 32 * n + 1] = fc_lr_w[g * HPC + n, :]
            fcbp[0, 32 * n] = fc_wd_b[g * HPC + n]
            fcbp[0, 32 * n + 1] = fc_lr_b[g * HPC + n]
            basep[32 * n, 0] = np.exp(log_base_weight_decay[g * HPC + n])
            basep[32 * n + 1, 0] = -np.exp(log_base_lr[g * HPC + n])
        fcp = np.ascontiguousarray(
            fcp.reshape(8, 128, 128).transpose(1, 0, 2).reshape(128, 1024),
            dtype=BFNP)
        qkvbT = np.zeros((128, 6), f)
        for j, bias in enumerate([q_b, k_b, v_b]):
            for mg in range(2):
                qkvbT[:, j * 2 + mg] = bias[cs][mg * 128:(mg + 1) * 128]
        w1 = W1[b, hs]
        w2 = W2[b, hs]
        w1t = np.ascontiguousarray(w1.transpose(0, 2, 1), dtype=BFNP)
        w2t = np.ascontiguousarray(w2.transpose(0, 2, 1), dtype=BFNP)
        owT = np.ascontiguousarray(o_w[:, cs].T, dtype=BFNP)
        in_maps.append({
            "xTb": xT.astype(BFNP), "qkvT": qkvT,
            "fcp": fcp, "fcbp": fcbp, "basep": basep, "qkvbT": qkvbT,
            "w1t": w1t, "w1n": np.ascontiguousarray(w1, dtype=f),
            "w2t": w2t, "w2nb": np.ascontiguousarray(w2, dtype=BFNP),
            "w2n": np.ascontiguousarray(w2, dtype=f),
            "owT": owT,
        })
    return in_maps


def run(inputs, trace=False):
    nc = _get_nc()
    in_maps = make_in_maps(**inputs)
    res = run_bass_kernel_spmd(nc, in_maps, list(range(NCORES)), trace=trace)
    out = np.zeros((B, L, D), np.float32)
    W1n = np.zeros((B, NH, HDH, HD), np.float32)
    W2n = np.zeros((B, NH, HD, HDH), np.float32)
    for core in range(NCORES):
        b, g = core // 4, core % 4
        r = res.results[core]
        out[b] += r["out_p"]
        W1n[b, g * HPC:(g + 1) * HPC] = r["w1next"]
        W2n[b, g * HPC:(g + 1) * HPC] = r["w2next"]
    out += np.asarray(inputs["o_b"], np.float32)[None, None, :]
    return (out, W1n, W2n), res


def kernel(**inputs):
    inputs = {k: np.asarray(v) for k, v in inputs.items()}
    (out, W1n, W2n), _ = run(inputs)
    return out, W1n, W2n


if __name__ == "__main__":
    print("building program...")
    nc = _get_nc()
    print("built ok")


# revision 35
# speedup vs baseline: 2.9882x; 1.0104x over previous
"""Trainium2 Bass kernel for nn_MultiHeadNeuralMemoryMLP.

Math reformulation (per batch b, head n), avoiding the [L, L] decay masks:
  cum[l]      = cumsum(log_wd)[l],  wd_cross[l] = exp(cum[l])
  wd_inner[l, m] = wd_cross[m] / wd_cross[l]  (for l <= m)
  nG1 = -grad_Z1 * (lr * exp(-cum))[:, None]   (lr sign folded in)
  Z1_[m] = wd_cross[m] * (S_masked.T @ nG1 + X1_ @ W1.T)[m],  S = X1 @ X1_.T causal
  Z2_[m] = wd_cross[m] * (S2_masked.T @ nG2 + X2_ @ W2.T)[m], S2 = X2 @ X2_.T causal
  W1_next = wd_cross[L-1] * (W1 + nG1.T @ X1);  W2_next similarly.

Sharding: core = b * 4 + g handles batch b, heads 4g..4g+3. Projections use
replicated (sliced) weights; only the o-projection needs a cross-core sum,
done on the host over 4 partial [L, D] tensors per batch.

Matmul operands are bf16 (f32 PSUM accumulation); the scalar pipeline
(fc projections, log-weight-decay cumsum, per-token scalars) stays f32.
The scalar engine runs only Sigmoid in steady state (silu/dsilu composed
on DVE) to avoid activation-table reloads.
"""

import sys

import numpy as np
import ml_dtypes

if "/opt/trn_rl_repo" not in sys.path:
    sys.path.insert(0, "/opt/trn_rl_repo")

import concourse.bass as bass
import concourse.mybir as mybir
import concourse.tile as tile
from concourse import bacc
from concourse.bass_utils import run_bass_kernel_spmd

F32 = mybir.dt.float32
BF16 = mybir.dt.bfloat16
AF = mybir.ActivationFunctionType
BFNP = ml_dtypes.bfloat16

B, L, D, NH, DH = 2, 1024, 1024, 16, 2048
HD, HDH = D // NH, DH // NH          # 64, 128
HPC = 4                               # heads per core
NCORES = 8
NLT = L // 128                        # 8 row tiles
NCH = L // 512                        # 2 column chunks


def build_program():
    nc = bacc.Bacc("TRN2", target_bir_lowering=False, debug=False,
                   num_devices=NCORES)

    # ---- DRAM I/O ----
    xTb_d = nc.dram_tensor("xTb", [D, L], BF16, kind="ExternalInput")
    qkvT_d = nc.dram_tensor("qkvT", [D, 3 * HPC * HD], BF16, kind="ExternalInput")
    fcp_d = nc.dram_tensor("fcp", [128, L], BF16, kind="ExternalInput")
    fcbp_d = nc.dram_tensor("fcbp", [1, 128], F32, kind="ExternalInput")
    basep_d = nc.dram_tensor("basep", [128, 1], F32, kind="ExternalInput")
    qkvbT_d = nc.dram_tensor("qkvbT", [128, 6], F32, kind="ExternalInput")
    w1t_d = nc.dram_tensor("w1t", [HPC, HD, HDH], BF16, kind="ExternalInput")
    w1n_d = nc.dram_tensor("w1n", [HPC, HDH, HD], F32, kind="ExternalInput")
    w2t_d = nc.dram_tensor("w2t", [HPC, HDH, HD], BF16, kind="ExternalInput")
    w2nb_d = nc.dram_tensor("w2nb", [HPC, HD, HDH], BF16, kind="ExternalInput")
    w2n_d = nc.dram_tensor("w2n", [HPC, HD, HDH], F32, kind="ExternalInput")
    owT_d = nc.dram_tensor("owT", [HPC * HD, D], BF16, kind="ExternalInput")

    out_d = nc.dram_tensor("out_p", [L, D], F32, kind="ExternalOutput")
    w1next_d = nc.dram_tensor("w1next", [HPC, HDH, HD], F32, kind="ExternalOutput")
    w2next_d = nc.dram_tensor("w2next", [HPC, HD, HDH], F32, kind="ExternalOutput")

    triu_d = nc.inline_tensor(np.triu(np.ones((128, 128), np.float32)), "triu")
    ident_d = nc.inline_tensor(np.eye(128, dtype=np.float32), "ident")
    identb_d = nc.inline_tensor(np.eye(128, dtype=BFNP), "identb")
    ones_d = nc.inline_tensor(np.ones((128, 512), np.float32), "ones")

    with tile.TileContext(nc) as tc:
        with (
            tc.tile_pool(name="big", bufs=8) as big,       # 4KB/partition slots
            tc.tile_pool(name="bigq", bufs=8) as bigq,
            tc.tile_pool(name="qkv", bufs=1) as qkvp,
            tc.tile_pool(name="const", bufs=1) as cst,
            tc.tile_pool(name="scal", bufs=1) as scp,
            tc.tile_pool(name="head", bufs=8) as hp,
            tc.tile_pool(name="head2", bufs=2) as hp2,
            tc.tile_pool(name="stile", bufs=16) as sp,
            tc.tile_pool(name="tmp", bufs=2) as tmp,
            tc.tile_pool(name="psA", bufs=3, space="PSUM") as psA,
            tc.tile_pool(name="psT", bufs=2, space="PSUM") as psT,
            tc.tile_pool(name="psB", bufs=2, space="PSUM") as psB,
            tc.tile_pool(name="psW", bufs=1, space="PSUM") as psW,
        ):
            dma = nc.sync.dma_start

            # ---- inputs to SBUF (fc weights + x first: they gate stage A) ----
            fcp = cst.tile([128, L], BF16, tag="fcp")
            fcbp = cst.tile([1, 128], F32, tag="fcbp")
            basep = cst.tile([128, 1], F32, tag="basep")
            qkvbT = cst.tile([128, 6], F32, tag="qkvbT")
            dma(fcp[:], fcp_d[:])
            dma(fcbp[:], fcbp_d[:])
            dma(basep[:], basep_d[:])
            dma(qkvbT[:], qkvbT_d[:])
            xtb, qk = [], []
            for kt in range(NLT):
                tb = bigq.tile([128, L], BF16, tag="xtb", name=f"xtb{kt}")
                dma(tb[:], xTb_d[kt * 128:(kt + 1) * 128, :])
                xtb.append(tb)
                tq = bigq.tile([128, 3 * HPC * HD], BF16, tag="qk",
                               name=f"qk{kt}")
                dma(tq[:], qkvT_d[kt * 128:(kt + 1) * 128, :])
                qk.append(tq)
            triu = cst.tile([128, 128], F32, tag="triu")
            ident = cst.tile([128, 128], F32, tag="ident")
            identb = cst.tile([128, 128], BF16, tag="identb")
            ones = cst.tile([128, 512], F32, tag="ones")
            dma(triu[:], triu_d[:])
            dma(ident[:], ident_d[:])
            dma(identb[:], identb_d[:])
            dma(ones[:], ones_d[:])
            owt = []
            for kt in range(2):
                t = bigq.tile([128, D], BF16, tag="owt", name=f"owt{kt}",
                              bufs=2)
                dma(t[:], owT_d[kt * 128:(kt + 1) * 128, :])
                owt.append(t)

            # ---- stage A: fc projections (T layout, head n mapped to
            # partition 32n (wd) / 32n+1 (lr)), log_wd, cumsum, scalars ----
            sigTp = scp.tile([128, L], F32, tag="sigTp")
            for ch in range(NCH):
                psfc = psA.tile([128, 512], F32, tag="psA")
                for kt in range(NLT):
                    nc.tensor.matmul(
                        psfc[:], fcp[:, kt * 128:(kt + 1) * 128],
                        xtb[kt][:, ch * 512:(ch + 1) * 512],
                        start=(kt == 0), stop=False)
                nc.tensor.matmul(psfc[:], fcbp[:], ones[0:1, 0:512],
                                 start=False, stop=True)
                nc.scalar.activation(sigTp[:, ch * 512:(ch + 1) * 512],
                                     psfc[:], AF.Sigmoid)
            sbTp = scp.tile([128, L], F32, tag="sbTp")
            nc.vector.tensor_scalar_mul(sbTp[:], sigTp[:], basep[:])
            for n in range(HPC):
                nc.scalar.activation(sbTp[32 * n:32 * n + 1, :],
                                     sbTp[32 * n:32 * n + 1, :],
                                     AF.Ln, bias=1.0, scale=-1.0)
            # natural tiles: nat[lt][:, 32n] = log_wd, [:, 32n+1] = -lr*sig
            nat = []
            for lt in range(NLT):
                psn = psB.tile([128, 128], F32, tag="psB")
                nc.tensor.transpose(psn[:], sbTp[:, lt * 128:(lt + 1) * 128],
                                    ident[:])
                nt = scp.tile([128, 128], F32, tag=f"nat{lt}")
                nc.scalar.copy(nt[:], psn[:])
                nat.append(nt)

            # cumT (T layout, heads at partitions 0/32/64/96)
            cumTp = scp.tile([128, L], F32, tag="cumTp")
            for mt in range(NLT):
                pscum = psB.tile([128, 128], F32, tag="psB")
                for lt in range(mt):
                    nc.tensor.matmul(pscum[:], nat[lt][:], ones[0:128, 0:128],
                                     start=(lt == 0), stop=False)
                nc.tensor.matmul(pscum[:], nat[mt][:], triu[:],
                                 start=(mt == 0), stop=True)
                nc.scalar.copy(cumTp[:, mt * 128:(mt + 1) * 128], pscum[:])
            wdcTp = scp.tile([128, L], F32, tag="wdcTp")
            nc.scalar.activation(wdcTp[:], cumTp[:], AF.Exp)

            # wd_last broadcast [128, 4] (col n = wd_cross[L-1] of head n)
            wdlast = scp.tile([128, HPC], F32, tag="wdlast")
            for n in range(HPC):
                pswl = psB.tile([128, 1], F32, tag="psB")
                nc.tensor.matmul(pswl[:], ones[32 * n:32 * n + 1, 0:128],
                                 wdcTp[32 * n:32 * n + 1, L - 1:L],
                                 tile_position=(32 * n, 0))
                nc.vector.tensor_copy(wdlast[:, n:n + 1], pswl[:])

            # natural-layout lrw[lt][:, n] = -lr[l] * exp(-cum[l])
            lrw = []
            for lt in range(NLT):
                psct = psB.tile([128, 128], F32, tag="psB")
                nc.tensor.transpose(psct[:], cumTp[:, lt * 128:(lt + 1) * 128],
                                    ident[:])
                lw_t = scp.tile([128, HPC], F32, tag=f"lrw{lt}")
                for n in range(HPC):
                    iw = tmp.tile([128, 1], F32, tag="iw")
                    nc.scalar.activation(iw[:], psct[:, 32 * n:32 * n + 1],
                                         AF.Exp, scale=-1.0)
                    nc.vector.tensor_mul(lw_t[:, n:n + 1], iw[:],
                                         nat[lt][:, 32 * n + 1:32 * n + 2])
                lrw.append(lw_t)

            # ---- stage B: q/k/v projections in T layout (bf16) ----
            qkv_t = []
            for j in range(3):
                pair_tiles = []
                for mg in range(2):
                    dst = qkvp.tile([128, L], BF16, tag=f"qkv{j}{mg}")
                    off = j * HPC * HD + mg * 128
                    for ch in range(NCH):
                        psp = psA.tile([128, 512], F32, tag="psA")
                        for kt in range(NLT):
                            nc.tensor.matmul(
                                psp[:], qk[kt][:, off:off + 128],
                                xtb[kt][:, ch * 512:(ch + 1) * 512],
                                start=(kt == 0), stop=(kt == NLT - 1))
                        nc.vector.tensor_scalar_add(
                            dst[:, ch * 512:(ch + 1) * 512], psp[:],
                            qkvbT[:, j * 2 + mg:j * 2 + mg + 1])
                    pair_tiles.append(dst)
                qkv_t.append(pair_tiles)
            Q2T, K2T, V2T = qkv_t

            # Z2catT: pair tile kt holds heads 2kt, 2kt+1 (rows 0:64 / 64:128)
            z2cat = [big.tile([128, L], BF16, tag="big", name=f"z2cat{i}")
                     for i in range(2)]

            # wdb broadcast tiles ([128, 512] rows = wd_cross), built upfront
            wdbs = {}
            for n in range(HPC):
                for ch in range(NCH):
                    m0, m1 = ch * 512, (ch + 1) * 512
                    wdb = hp2.tile([128, 512], F32, tag="wdb", bufs=8,
                                   name=f"wdb{n}{ch}")
                    pswb = psA.tile([128, 512], F32, tag="psA")
                    nc.tensor.matmul(pswb[:], ones[32 * n:32 * n + 1, 0:128],
                                     wdcTp[32 * n:32 * n + 1, m0:m1],
                                     tile_position=(32 * n, 0))
                    nc.scalar.copy(wdb[:], pswb[:])
                    wdbs[(n, ch)] = wdb

            # ---- stage C/D: heads processed in pairs, stage-major so the
            # static schedule interleaves the two heads' chains ----
            for pair in range(2):
                heads = [2 * pair, 2 * pair + 1]
                st_ = {}
                for n in heads:
                    bp = (n % 2) * 64
                    w1t_s = hp2.tile([128, HDH], BF16, tag="w1t",
                                     name=f"w1t{n}")
                    dma(w1t_s[0:64, :], w1t_d[n])
                    dma(w1t_s[64:128, :], w1t_d[n])
                    w1n_s = hp2.tile([HDH, HD], F32, tag="w1n", name=f"w1n{n}")
                    dma(w1n_s[:], w1n_d[n])
                    w2t_s = hp2.tile([HDH, HD], BF16, tag="w2t", name=f"w2t{n}")
                    dma(w2t_s[:], w2t_d[n])
                    w2nb_s = hp2.tile([HD, HDH], BF16, tag="w2nb",
                                      name=f"w2nb{n}")
                    dma(w2nb_s[:], w2nb_d[n])
                    w2n_s = hp2.tile([HD, HDH], F32, tag="w2n", name=f"w2n{n}")
                    dma(w2n_s[:], w2n_d[n])
                    st_[n] = dict(
                        bp=bp, w1t_s=w1t_s, w1n_s=w1n_s, w2t_s=w2t_s,
                        w2nb_s=w2nb_s, w2n_s=w2n_s,
                        X1T=K2T[pair][bp:bp + 64, :],
                        X1_T=Q2T[pair][bp:bp + 64, :],
                        VT=V2T[pair][bp:bp + 64, :],
                        X2T=big.tile([128, L], BF16, tag="big", name=f"X2T{n}"),
                        X2_T=big.tile([128, L], BF16, tag="big",
                                      name=f"X2_T{n}"),
                        gZ2T=hp2.tile([HD, L], BF16, tag="gz2t",
                                      name=f"gz2t{n}"),
                        x2nh=[], dslh=[], nG1h=[], nG2h=[], X1nh=[])

                # fwd stage 1: Z1 -> sigmoid -> silu/dsilu (per half, per head)
                for hf in range(2):
                    for n in heads:
                        s = st_[n]
                        bp, h0 = s["bp"], hf * 512
                        psZ = psA.tile([128, 512], F32, tag="psA")
                        for j in range(4):
                            c0 = h0 + j * 128
                            nc.tensor.matmul(psZ[:, j * 128:(j + 1) * 128],
                                             s["X1T"][:, c0:c0 + 128],
                                             s["w1t_s"][bp:bp + 64, :])
                        sg = tmp.tile([128, 512], F32, tag="sg")
                        nc.scalar.activation(sg[:], psZ[:], AF.Sigmoid)
                        z1s = tmp.tile([128, 512], F32, tag="z1s")
                        nc.scalar.copy(z1s[:], psZ[:])
                        xa = hp.tile([128, 512], BF16, tag="x2n", bufs=8)
                        nc.vector.tensor_mul(xa[:], z1s[:], sg[:])   # silu
                        s["x2nh"].append(xa)
                        # dsilu = sil + sg*(1 - sil)
                        w_ = tmp.tile([128, 512], F32, tag="w_")
                        nc.vector.tensor_scalar(w_[:], xa[:], -1.0, 1.0,
                                                mybir.AluOpType.mult,
                                                mybir.AluOpType.add)
                        da = hp.tile([128, 512], BF16, tag="dsl", bufs=8)
                        nc.vector.scalar_tensor_tensor(
                            da[:], sg[:], 1.0, w_[:],
                            mybir.AluOpType.mult, mybir.AluOpType.mult)
                        nc.vector.tensor_add(da[:], da[:], xa[:])
                        s["dslh"].append(da)

                # fwd stage 2: X2T transposes
                for hf in range(2):
                    for n in heads:
                        s = st_[n]
                        h0 = hf * 512
                        psX = psA.tile([128, 512], BF16, tag="psA", name="psX")
                        for j in range(4):
                            nc.tensor.transpose(
                                psX[:, j * 128:(j + 1) * 128],
                                s["x2nh"][hf][:, j * 128:(j + 1) * 128],
                                identb[:])
                        nc.scalar.copy(s["X2T"][:, h0:h0 + 512], psX[:])

                # fwd stage 3: Z2T + grad_Z2
                for n in heads:
                    s = st_[n]
                    for ch in range(NCH):
                        ps3 = psA.tile([HD, 512], F32, tag="psA")
                        nc.tensor.matmul(ps3[:], s["w2t_s"][:],
                                         s["X2T"][:, ch * 512:(ch + 1) * 512])
                        nc.vector.tensor_sub(
                            s["gZ2T"][:, ch * 512:(ch + 1) * 512], ps3[:],
                            s["VT"][:, ch * 512:(ch + 1) * 512])

                # fwd stage 4: grad_X2, nG1/nG2, X1 natural
                for hf in range(2):
                    for n in heads:
                        s = st_[n]
                        bp, h0 = s["bp"], hf * 512
                        psG = psA.tile([128, 512], F32, tag="psA")
                        for j in range(4):
                            c0 = h0 + j * 128
                            nc.tensor.matmul(psG[:, j * 128:(j + 1) * 128],
                                             s["gZ2T"][:, c0:c0 + 128],
                                             s["w2nb_s"][:])
                        t1 = tmp.tile([128, 512], F32, tag="t1")
                        nc.vector.tensor_mul(t1[:], psG[:], s["dslh"][hf][:])
                        g1 = hp.tile([128, 512], BF16, tag="ng1", bufs=8)
                        for j in range(4):
                            lt = hf * 4 + j
                            nc.vector.tensor_scalar_mul(
                                g1[:, j * 128:(j + 1) * 128],
                                t1[:, j * 128:(j + 1) * 128],
                                lrw[lt][:, n:n + 1])
                        s["nG1h"].append(g1)
                        psU = psA.tile([128, 256], BF16, tag="psA", name="psU")
                        for j in range(4):
                            c0 = h0 + j * 128
                            nc.tensor.transpose(psU[:, j * 64:(j + 1) * 64],
                                                s["gZ2T"][:, c0:c0 + 128],
                                                identb[0:64, 0:64])
                        g2 = hp.tile([128, 256], BF16, tag="ng2", bufs=8)
                        for j in range(4):
                            lt = hf * 4 + j
                            nc.vector.tensor_scalar_mul(
                                g2[:, j * 64:(j + 1) * 64],
                                psU[:, j * 64:(j + 1) * 64],
                                lrw[lt][:, n:n + 1])
                        s["nG2h"].append(g2)
                        psV = psA.tile([128, 256], BF16, tag="psA", name="psV")
                        for j in range(4):
                            c0 = h0 + j * 128
                            nc.tensor.transpose(
                                psV[:, j * 64:(j + 1) * 64],
                                s["X1T"][:, c0:c0 + 128],
                                identb[bp:bp + 64, bp:bp + 64])
                        x1 = hp.tile([128, 256], BF16, tag="x1n", bufs=8)
                        nc.scalar.copy(x1[:], psV[:])
                        s["X1nh"].append(x1)

                # readout: interleave the two heads per 512-chunk
                for ch in range(NCH):
                    m0, m1 = ch * 512, (ch + 1) * 512
                    mb0 = 4 * ch
                    nlts = 4 * ch + 4

                    for n in heads:
                        s = st_[n]
                        bp = (n % 2) * 64
                        X1T = K2T[pair][bp:bp + 64, :]
                        X1_T = Q2T[pair][bp:bp + 64, :]

                        s["wdb"] = wdbs[(n, ch)]

                        S = []
                        for lt in range(nlts):
                            j0 = 0 if lt < mb0 else (lt - mb0) * 128
                            psS = psA.tile([128, 512], F32, tag="psA")
                            nc.tensor.matmul(psS[:, j0:512],
                                             X1T[:, lt * 128:(lt + 1) * 128],
                                             X1_T[:, m0 + j0:m1])
                            st = sp.tile([128, 512], BF16, tag="s")
                            if lt < mb0:
                                nc.scalar.copy(st[:], psS[:])
                            else:
                                nc.vector.tensor_mul(st[:, j0:j0 + 128],
                                                     psS[:, j0:j0 + 128], triu[:])
                                if j0 + 128 < 512:
                                    nc.scalar.copy(st[:, j0 + 128:512],
                                                   psS[:, j0 + 128:512])
                            S.append(st)

                        psT1 = psT.tile([128, 512], F32, tag="psT")
                        for lt in range(nlts):
                            j0 = 0 if lt < mb0 else (lt - mb0) * 128
                            nc.tensor.matmul(psT1[:, j0:512],
                                             s["nG1h"][lt // 4]
                                             [:, (lt % 4) * 128:(lt % 4 + 1) * 128],
                                             S[lt][:, j0:512],
                                             start=(lt == 0), stop=False)
                        nc.tensor.matmul(psT1[:], s["w1t_s"][bp:bp + 64, :],
                                         X1_T[:, m0:m1], start=False, stop=True)
                        z1t = hp2.tile([128, 512], F32, tag="z1t", bufs=4)
                        nc.vector.tensor_mul(z1t[:], psT1[:], s["wdb"][:])
                        sgro = tmp.tile([128, 512], F32, tag="sgro")
                        nc.scalar.activation(sgro[:], z1t[:], AF.Sigmoid)
                        nc.vector.tensor_mul(s["X2_T"][:, m0:m1], z1t[:], sgro[:])

                    for n in heads:
                        s = st_[n]
                        bp = (n % 2) * 64

                        S2 = []
                        for lt in range(nlts):
                            j0 = 0 if lt < mb0 else (lt - mb0) * 128
                            psS2 = psA.tile([128, 512], F32, tag="psA")
                            nc.tensor.matmul(psS2[:, j0:512],
                                             s["X2T"][:, lt * 128:(lt + 1) * 128],
                                             s["X2_T"][:, m0 + j0:m1])
                            st = sp.tile([128, 512], BF16, tag="s", name="s2t")
                            if lt < mb0:
                                nc.scalar.copy(st[:], psS2[:])
                            else:
                                nc.vector.tensor_mul(st[:, j0:j0 + 128],
                                                     psS2[:, j0:j0 + 128],
                                                     triu[:])
                                if j0 + 128 < 512:
                                    nc.scalar.copy(st[:, j0 + 128:512],
                                                   psS2[:, j0 + 128:512])
                            S2.append(st)

                        psT2 = psT.tile([HD, 512], F32, tag="psT")
                        for lt in range(nlts):
                            j0 = 0 if lt < mb0 else (lt - mb0) * 128
                            nc.tensor.matmul(psT2[:, j0:512],
                                             s["nG2h"][lt // 4]
                                             [:, (lt % 4) * 64:(lt % 4 + 1) * 64],
                                             S2[lt][:, j0:512],
                                             start=(lt == 0), stop=False)
                        nc.tensor.matmul(psT2[:], s["w2t_s"][:],
                                         s["X2_T"][:, m0:m1],
                                         start=False, stop=True)
                        nc.vector.tensor_mul(z2cat[pair][bp:bp + 64, m0:m1],
                                             psT2[:], s["wdb"][0:64, :])

                # weight updates (off the readout critical path)
                for n in heads:
                    s = st_[n]
                    psw1 = psW.tile([HDH, HD], F32, tag="psW")
                    for lt in range(NLT):
                        hf, j = lt // 4, lt % 4
                        nc.tensor.matmul(psw1[:],
                                         s["nG1h"][hf][:, j * 128:(j + 1) * 128],
                                         s["X1nh"][hf][:, j * 64:(j + 1) * 64],
                                         start=(lt == 0), stop=(lt == NLT - 1))
                    tw1 = tmp.tile([HDH, HD], F32, tag="tw1")
                    nc.vector.tensor_add(tw1[:], psw1[:], s["w1n_s"][:])
                    nc.vector.tensor_scalar_mul(tw1[:], tw1[:],
                                                wdlast[:, n:n + 1])
                    dma(w1next_d[n], tw1[:])
                    psw2 = psW.tile([HD, HDH], F32, tag="psW")
                    for lt in range(NLT):
                        hf, j = lt // 4, lt % 4
                        nc.tensor.matmul(psw2[:],
                                         s["nG2h"][hf][:, j * 64:(j + 1) * 64],
                                         s["x2nh"][hf][:, j * 128:(j + 1) * 128],
                                         start=(lt == 0), stop=(lt == NLT - 1))
                    tw2 = tmp.tile([HD, HDH], F32, tag="tw2")
                    nc.vector.tensor_add(tw2[:], psw2[:], s["w2n_s"][0:64, :])
                    nc.vector.tensor_scalar_mul(tw2[:], tw2[:],
                                                wdlast[0:64, n:n + 1])
                    dma(w2next_d[n], tw2[:])

            # ---- stage E: o-projection (partial, heads of this core) ----
            for lt in range(NLT):
                outs = big.tile([128, D], F32, tag="big", name=f"outs{lt}")
                for ch in range(NCH):
                    psO = psA.tile([128, 512], F32, tag="psA")
                    for kt in range(2):
                        nc.tensor.matmul(
                            psO[:], z2cat[kt][:, lt * 128:(lt + 1) * 128],
                            owt[kt][:, ch * 512:(ch + 1) * 512],
                            start=(kt == 0), stop=(kt == 1))
                    if ch == 0:
                        nc.scalar.copy(outs[:, ch * 512:(ch + 1) * 512], psO[:])
                    else:
                        nc.vector.tensor_copy(outs[:, ch * 512:(ch + 1) * 512],
                                              psO[:])
                dma(out_d[lt * 128:(lt + 1) * 128, :], outs[:])

    nc.compile()
    return nc


_NC = None


def _get_nc():
    global _NC
    if _NC is None:
        _NC = build_program()
    return _NC


def make_in_maps(x, W1, W2, log_base_lr, fc_lr_w, fc_lr_b, log_base_weight_decay,
                 fc_wd_w, fc_wd_b, q_w, q_b, k_w, k_b, v_w, v_b, o_w, o_b):
    f = np.float32
    in_maps = []
    for core in range(NCORES):
        b, g = core // 4, core % 4
        hs = slice(g * HPC, (g + 1) * HPC)
        cs = slice(g * HPC * HD, (g + 1) * HPC * HD)
        xT = np.ascontiguousarray(x[b].T, dtype=f)
        qkvT = np.ascontiguousarray(
            np.concatenate([q_w[cs, :].T, k_w[cs, :].T, v_w[cs, :].T], axis=1),
            dtype=BFNP)
        # fc weights padded: within each 128-col k-block, col 32n = wd head n,
        # col 32n+1 = lr head n (others zero)
        fcp = np.zeros((D, 128), f)
        fcbp = np.zeros((1, 128), f)
        basep = np.zeros((128, 1), f)
        for n in range(HPC):
            fcp[:, 32 * n] = fc_wd_w[g * HPC + n, :]
            fcp[:, 32 * n + 1] = fc_lr_w[g * HPC + n, :]
            fcbp[0, 32 * n] = fc_wd_b[g * HPC + n]
            fcbp[0, 32 * n + 1] = fc_lr_b[g * HPC + n]
            basep[32 * n, 0] = np.exp(log_base_weight_decay[g * HPC + n])
            basep[32 * n + 1, 0] = -np.exp(log_base_lr[g * HPC + n])
        fcp = np.ascontiguousarray(
            fcp.reshape(8, 128, 128).transpose(1, 0, 2).reshape(128, 1024),
            dtype=BFNP)
        qkvbT = np.zeros((128, 6), f)
        for j, bias in enumerate([q_b, k_b, v_b]):
            for mg in range(2):
                qkvbT[:, j * 2 + mg] = bias[cs][mg * 128:(mg + 1) * 128]
        w1 = W1[b, hs]
        w2 = W2[b, hs]
        w1t = np.ascontiguousarray(w1.transpose(0, 2, 1), dtype=BFNP)
        w2t = np.ascontiguousarray(w2.transpose(0, 2, 1), dtype=BFNP)
        owT = np.ascontiguousarray(o_w[:, cs].T, dtype=BFNP)
        in_maps.append({
            "xTb": xT.astype(BFNP), "qkvT": qkvT,
            "fcp": fcp, "fcbp": fcbp, "basep": basep, "qkvbT": qkvbT,
            "w1t": w1t, "w1n": np.ascontiguousarray(w1, dtype=f),
            "w2t": w2t, "w2nb": np.ascontiguousarray(w2, dtype=BFNP),
            "w2n": np.ascontiguousarray(w2, dtype=f),
            "owT": owT,
        })
    return in_maps


def run(inputs, trace=False):
    nc = _get_nc()
    in_maps = make_in_maps(**inputs)
    res = run_bass_kernel_spmd(nc, in_maps, list(range(NCORES)), trace=trace)
    out = np.zeros((B, L, D), np.float32)
    W1n = np.zeros((B, NH, HDH, HD), np.float32)
    W2n = np.zeros((B, NH, HD, HDH), np.float32)
    for core in range(NCORES):
        b, g = core // 4, core % 4
        r = res.results[core]
        out[b] += r["out_p"]
        W1n[b, g * HPC:(g + 1) * HPC] = r["w1next"]
        W2n[b, g * HPC:(g + 1) * HPC] = r["w2next"]
    out += np.asarray(inputs["o_b"], np.float32)[None, None, :]
    return (out, W1n, W2n), res


def kernel(**inputs):
    inputs = {k: np.asarray(v) for k, v in inputs.items()}
    (out, W1n, W2n), _ = run(inputs)
    return out, W1n, W2n


if __name__ == "__main__":
    print("building program...")
    nc = _get_nc()
    print("built ok")
